# revision 37
# baseline (speedup 1.0000x reference)
"""DenoiseLSTM Trainium2 kernel (8 NeuronCores, SPMD, batch-parallel).

Strategy: fully data-parallel over batch — each core runs the ENTIRE network
(bi-LSTM encoder, LSTM decoder with attention, FFN, vocab projection) for its
4 of the 32 batches. No collectives; the host concatenates batch shards.

Key optimizations over the vocab-sharded baseline:
- All elementwise/softmax/FFN work shrinks 8x per core (B_local=4), so the
  LSTM steps' serial DVE/ACT chain is ~2.6x shorter per step.
- Recurrence weights quantized to fp8e4 (x sw=32); hidden state kept in
  fp16 (x sh=128) — mixed fp8-lhsT x fp16-rhs matmuls are HW-verified.
  fp8 weights halve the per-step LDWEIGHTS stream on hardware (FWL reads
  4 fp8/cycle), which is the decoder's throughput floor and is NOT modeled
  by the CoreSim cost model. Descale (1/(sw*sh)) is folded into the gate
  activations' `scale` operand.
- The x-projection contribution enters the gate PSUM through one extra wide
  matmul per step (lhsT = (sw*sh)*Identity, rhs = the step's whole xproj
  block), removing both the DVE add and 15 weight reloads from the step.
  PSUM zero-region semantics: only the first matmul per gate tile carries
  start=True, and gate PSUM tiles are padded to a full 2KB bank.
- Gate layout reordered to [i, f, o, g] so one sigmoid covers i,f,o and the
  tanh(g) tail is minimal; cell ops run on fp16 SBUF tiles (4x DVE mode);
  off-critical stores (h->memory/H) run on the otherwise idle GPSIMD
  (SBUF-only: GPSIMD cannot access PSUM).
- Token/style embedding gathers are host-side indexing; no device gathers.
- Vocab projection streams the full W_f2 [32000,512] once (bf16) while
  accumulating bf16 logits in SBUF (PE ~99% busy at the N-column roofline);
  wf2-in DMAs ride the sync queue, logits-out the scalar queue; output is
  bf16 on device and upcast to f32 on host.
"""
import sys

sys.path.insert(0, "/opt/trn_rl_repo")

from contextlib import ExitStack

import numpy as np
import ml_dtypes

import concourse.bass as bass
import concourse.bacc as bacc
import concourse.mybir as mybir
import concourse.tile as tile
from concourse.bass_utils import run_bass_kernel_spmd
from concourse.masks import make_identity

bf16 = ml_dtypes.bfloat16
fp16 = np.float16
fp8 = ml_dtypes.float8_e4m3
F32 = mybir.dt.float32
BF16 = mybir.dt.bfloat16
FP16 = mybir.dt.float16
F8 = mybir.dt.float8e4
AF = mybir.ActivationFunctionType
ALU = mybir.AluOpType
AX = mybir.AxisListType

B = 32
BL = 4                 # batches per core
D_EMB = 128
D_ENC = 256
D_DEC = 512
N_CORES = 8
S = 128
T = 128
V = 32000

KD = 4                 # decoder hidden chunks (512/128)
MD = 16                # decoder gate chunks (2048/128)
KE = 2                 # encoder hidden chunks per direction
QE = 16                # encoder fused gate slabs: gate(4) x dir(2) x j(2)
SH = 128.0             # hidden-state fp8 scale (compile-time)

VQ = 4000              # vocab output block
WC = 1000              # wf2 streaming chunk (v columns)


class _Stop(Exception):
    pass


def build(phases=7, use_fp8=True, fp16_h=True, bias_free=True):
    nc = bacc.Bacc("TRN2", target_bir_lowering=False, debug=False)
    WDT = F8 if use_fp8 else BF16
    HDT = FP16 if fp16_h else WDT
    SCALE = 1.0 / float(np.sqrt(np.float32(2 * D_ENC)))

    # ---- external inputs (host-prepped, per core) ----
    embT_e = nc.dram_tensor("embT_e", [128, S * BL], BF16, kind="ExternalInput")
    embT_d = nc.dram_tensor("embT_d", [128, T * BL], BF16, kind="ExternalInput")
    h0e_q = nc.dram_tensor("h0e_q", [128, 2 * KE * BL], HDT, kind="ExternalInput")
    ht0_q = nc.dram_tensor("ht0_q", [128, KD * BL], HDT, kind="ExternalInput")
    wih_e = nc.dram_tensor("wih_e", [128, QE * 128], BF16, kind="ExternalInput")
    wih_d = nc.dram_tensor("wih_d", [128, MD * 128], BF16, kind="ExternalInput")
    whh_e = nc.dram_tensor("whh_e", [128, QE * KE * 128], WDT, kind="ExternalInput")
    whh_d = nc.dram_tensor("whh_d", [128, MD * KD * 128], WDT, kind="ExternalInput")
    wtr = nc.dram_tensor("wtr", [128, KD * KD * 128], FP16, kind="ExternalInput")
    wf1 = nc.dram_tensor("wf1", [128, 4 * 8 * 128], BF16, kind="ExternalInput")
    wf2 = nc.dram_tensor("wf2", [128, KD * V], BF16, kind="ExternalInput")
    bs_e = nc.dram_tensor("bs_e", [128, QE], F32, kind="ExternalInput")
    bs_d = nc.dram_tensor("bs_d", [128, MD], F32, kind="ExternalInput")
    b1a = nc.dram_tensor("b1a", [128, KD], F32, kind="ExternalInput")
    b1h = nc.dram_tensor("b1h", [128, KD], F32, kind="ExternalInput")
    descale = nc.dram_tensor("descale", [128, 1], F32, kind="ExternalInput")
    idscale = nc.dram_tensor("idscale", [128, 1], F32, kind="ExternalInput")

    out = nc.dram_tensor("out", [BL, T, V], BF16, kind="ExternalOutput")

    with tile.TileContext(nc) as tc, ExitStack() as ctx:
        wpool = ctx.enter_context(tc.tile_pool(name="weights", bufs=1))
        spool = ctx.enter_context(tc.tile_pool(name="state", bufs=1))
        big = ctx.enter_context(tc.tile_pool(name="big", bufs=1))

        def load(dram, shape, dtype, tag):
            t = wpool.tile(shape, dtype, tag=tag, name=tag)
            nc.sync.dma_start(t[:], dram[:, :])
            return t

        embT_e_s = load(embT_e, [128, S * BL], BF16, "embT_e")
        embT_d_s = load(embT_d, [128, T * BL], BF16, "embT_d")
        wih_e_s = load(wih_e, [128, QE * 128], BF16, "wih_e")
        wih_d_s = load(wih_d, [128, MD * 128], BF16, "wih_d")
        whh_e_s = load(whh_e, [128, QE * KE * 128], WDT, "whh_e")
        whh_d_s = load(whh_d, [128, MD * KD * 128], WDT, "whh_d")
        wtr_s = load(wtr, [128, KD * KD * 128], FP16, "wtr")
        wf1_s = load(wf1, [128, 4 * 8 * 128], BF16, "wf1")
        bs_e_s = load(bs_e, [128, QE], F32, "bs_e")
        bs_d_s = load(bs_d, [128, MD], F32, "bs_d")
        b1a_s = load(b1a, [128, KD], F32, "b1a")
        b1h_s = load(b1h, [128, KD], F32, "b1h")
        desc_s = load(descale, [128, 1], F32, "descale")
        idsc_s = load(idscale, [128, 1], F32, "idscale")

        ident = wpool.tile([128, 128], BF16, tag="ident", name="ident")
        make_identity(nc, ident)
        # identity scaled by sw*sh: x-projection enters PSUM pre-scaled so the
        # gate activation descale (1/(sw*sh)) recovers the true pre-activation
        id_q = wpool.tile([128, 128], BF16, tag="id_q", name="id_q")
        nc.scalar.activation(id_q[:], ident[:], AF.Copy, scale=idsc_s[:, 0:1])

        # ---- state tiles ----
        h_e = [spool.tile([128, 2 * KE * BL], HDT, tag=f"h_e{j}", name=f"h_e{j}")
               for j in range(2)]
        c_e = spool.tile([128, 2 * KE * BL], FP16, tag="c_e", name="c_e")
        h_d = [spool.tile([128, KD * BL], HDT, tag=f"h_d{j}", name=f"h_d{j}")
               for j in range(2)]
        c_d = spool.tile([128, KD * BL], FP16, tag="c_d", name="c_d")
        nc.sync.dma_start(h_e[0][:], h0e_q[:, :])
        nc.sync.dma_start(h_d[0][:], ht0_q[:, :])
        nc.vector.memset(c_e[:], 0.0)

        # xproj: [128, s, q, b] so a step's rhs block is [128, q, b]
        xp_e = big.tile([128, S, QE, BL], BF16, tag="xp_e", name="xp_e")
        xp_d = big.tile([128, T, MD, BL], BF16, tag="xp_d", name="xp_d")
        # memory (encoder h) and decoder h, [128, kd, b, s/t]
        mem_T = big.tile([128, 2 * KE, BL, S], BF16, tag="mem_T", name="mem_T")
        H_T = big.tile([128, KD, BL, T], BF16, tag="H_T", name="H_T")

        # ---------- phase 1: encoder x-projection ----------
        # (decoder x-projection is emitted after the encoder loop so the
        # scheduler can run it inside encoder PE/DVE gaps)
        with tc.tile_pool(name="xp_ps", bufs=4, space="PSUM") as xps:
            def proj(w_s, src, n_q, bias_s, dst):
                for q in range(n_q):
                    ps = xps.tile([128, 512], F32, tag="xp", name="xp")
                    nc.tensor.matmul(ps[:], w_s[:, q * 128:(q + 1) * 128],
                                     src[:], start=True, stop=True)
                    ps3 = ps[:].rearrange("p (s b) -> p s b", b=BL)
                    if bias_free and q % 2 == 0:
                        nc.scalar.copy(dst[:, :, q, :], ps3)
                    elif bias_free:
                        nc.vector.tensor_copy(dst[:, :, q, :], ps3)
                    else:
                        nc.vector.tensor_scalar_add(dst[:, :, q, :], ps3,
                                                    bias_s[:, q:q + 1])

            proj(wih_e_s, embT_e_s, QE, bs_e_s, xp_e)

        try:
            if phases < 2:
                raise _Stop
            # ---------- phase 2: encoder recurrence (dirs fused) ----------
            # slab q = gate*4 + dir*2 + j;  gates ordered [i, f, o, g]
            # gate tile cols: i [0:16], f [16:32], o [32:48], g [48:64]
            # h/c cols: dir*8 + k*4 + b
            GW = 16
            with tc.tile_pool(name="enc_ps", bufs=2, space="PSUM") as eps, \
                 tc.tile_pool(name="enc_g", bufs=2) as egp:
                for step in range(S):
                    hcur = h_e[step % 2]
                    hnxt = h_e[(step + 1) % 2]
                    # full 2KB bank per gate tile: a start=True marks the
                    # whole PSUM zero-region, so tiles must not share banks
                    psf = eps.tile([128, 512], F32, tag="g", name="g")
                    ps = psf[:, 0:4 * GW]
                    # x-side: one matmul per (gate, dir) — contiguous 8-col
                    # out slabs; the first start=True marks the zero-region
                    for gv in range(4):
                        for dirv in range(2):
                            s_idx = step if dirv == 0 else S - 1 - step
                            q0 = gv * 4 + dirv * 2
                            nc.tensor.matmul(
                                ps[:, q0 * BL:(q0 + 2) * BL], id_q[:],
                                xp_e[:, s_idx, q0:q0 + 2, :],
                                start=(gv == 0 and dirv == 0), stop=False,
                                skip_group_check=True)
                    for q in range(QE):
                        d = (q >> 1) & 1
                        sl = ps[:, q * BL:(q + 1) * BL]
                        for k in range(KE):
                            rhs = hcur[:, (d * KE + k) * BL:(d * KE + k + 1) * BL]
                            lt = whh_e_s[:, (q * KE + k) * 128:(q * KE + k + 1) * 128]
                            nc.tensor.matmul(sl, lt, rhs,
                                             start=False, stop=(k == KE - 1),
                                             skip_group_check=True)
                    g = egp.tile([128, 4 * GW], FP16, tag="gg", name="gg")
                    nc.scalar.activation(g[:, 0:3 * GW], ps[:, 0:3 * GW],
                                         AF.Sigmoid, scale=desc_s[:, 0:1])
                    nc.scalar.activation(g[:, 3 * GW:4 * GW], ps[:, 3 * GW:4 * GW],
                                         AF.Tanh, scale=desc_s[:, 0:1])
                    t1 = egp.tile([128, GW], FP16, tag="t1", name="t1")
                    nc.vector.tensor_tensor(t1[:], g[:, GW:2 * GW], c_e[:], ALU.mult)
                    t2 = egp.tile([128, GW], FP16, tag="t2", name="t2")
                    nc.vector.tensor_tensor(t2[:], g[:, 0:GW], g[:, 3 * GW:4 * GW],
                                            ALU.mult)
                    nc.vector.tensor_tensor(c_e[:], t1[:], t2[:], ALU.add)
                    tc_t = egp.tile([128, GW], FP16, tag="tc", name="tc")
                    nc.scalar.activation(tc_t[:], c_e[:], AF.Tanh)
                    nc.vector.scalar_tensor_tensor(hnxt[:], g[:, 2 * GW:3 * GW],
                                                   SH, tc_t[:], ALU.mult, ALU.mult)
                    hb = egp.tile([128, GW], BF16, tag="hb", name="hb")
                    nc.gpsimd.tensor_tensor(hb[:], g[:, 2 * GW:3 * GW], tc_t[:],
                                            ALU.mult)
                    nc.gpsimd.tensor_copy(
                        mem_T[:, 0:KE, :, step],
                        hb[:, 0:KE * BL].rearrange("p (k b) -> p k b", k=KE))
                    nc.gpsimd.tensor_copy(
                        mem_T[:, KE:2 * KE, :, S - 1 - step],
                        hb[:, KE * BL:2 * KE * BL].rearrange("p (k b) -> p k b", k=KE))

            # decoder x-projection (independent of the encoder: fills gaps)
            with tc.tile_pool(name="xpd_ps", bufs=4, space="PSUM") as xps2:
                for q in range(MD):
                    ps = xps2.tile([128, 512], F32, tag="xpd", name="xpd")
                    nc.tensor.matmul(ps[:], wih_d_s[:, q * 128:(q + 1) * 128],
                                     embT_d_s[:], start=True, stop=True)
                    ps3 = ps[:].rearrange("p (s b) -> p s b", b=BL)
                    if bias_free and q % 2 == 0:
                        nc.scalar.copy(xp_d[:, :, q, :], ps3)
                    elif bias_free:
                        nc.vector.tensor_copy(xp_d[:, :, q, :], ps3)
                    else:
                        nc.vector.tensor_scalar_add(xp_d[:, :, q, :], ps3,
                                                    bs_d_s[:, q:q + 1])

            if phases < 3:
                raise _Stop
            # ---------- phase 3: c_t init (W_tr @ [cf;cb], lrelu) ----------
            with tc.tile_pool(name="ct_ps", bufs=2, space="PSUM") as cps, \
                 tc.tile_pool(name="ct_sb", bufs=2) as csb:
                for m in range(KD):
                    ps = cps.tile([128, BL], F32, tag="ct", name="ct")
                    for k in range(KD):
                        lt = wtr_s[:, (m * KD + k) * 128:(m * KD + k + 1) * 128]
                        nc.tensor.matmul(ps[:], lt, c_e[:, k * BL:(k + 1) * BL],
                                         start=(k == 0), stop=(k == KD - 1))
                    ab = csb.tile([128, BL], F32, tag="ab", name="ab")
                    nc.scalar.activation(ab[:], ps[:], AF.Abs)
                    idt = csb.tile([128, BL], F32, tag="idt", name="idt")
                    nc.scalar.activation(idt[:], ps[:], AF.Identity, scale=0.55)
                    nc.vector.scalar_tensor_tensor(c_d[:, m * BL:(m + 1) * BL],
                                                   ab[:], 0.45, idt[:],
                                                   ALU.mult, ALU.add)

            if phases < 4:
                raise _Stop
            # mem_norm transposes for attention (only need mem_T): spread
            # through the decoder loop to fill PE/ACT gaps
            memN = big.tile([S, BL, 2 * KE, 128], BF16, tag="memN", name="memN")

            # ---------- phase 4: decoder recurrence ----------
            # slab m = gate*4 + j; gates [i, f, o, g]; h/c cols k*4+b
            GW = KD * BL  # 16
            with tc.tile_pool(name="dec_ps", bufs=2, space="PSUM") as dps, \
                 tc.tile_pool(name="dec_g", bufs=2) as dgp, \
                 tc.tile_pool(name="mn_ps", bufs=2, space="PSUM") as mnpool:
                for step in range(T):
                    if step % 8 == 4 and step // 8 < 2 * KE * BL:
                        idx = step // 8
                        b, kd = idx // (2 * KE), idx % (2 * KE)
                        mn_ps = mnpool.tile([S, 128], BF16, tag="mn", name="mn",
                                            padded_shape=[128, 128])
                        nc.tensor.transpose(mn_ps[:], mem_T[:, kd, b, :],
                                            ident[:])
                        nc.scalar.copy(memN[:, b, kd, :], mn_ps[:])
                    hcur = h_d[step % 2]
                    hnxt = h_d[(step + 1) % 2]
                    psf = dps.tile([128, 512], F32, tag="gd", name="gd")
                    ps = psf[:, 0:4 * GW]
                    # x-side: single wide matmul (rhs = whole step's xproj
                    # block); start=True marks the tile's zero-region once
                    nc.tensor.matmul(ps[:], id_q[:], xp_d[:, step, :, :],
                                     start=True, stop=False,
                                     skip_group_check=True)
                    for m in range(MD):
                        sl = ps[:, m * BL:(m + 1) * BL]
                        for k in range(KD):
                            rhs = hcur[:, k * BL:(k + 1) * BL]
                            lt = whh_d_s[:, (m * KD + k) * 128:
                                         (m * KD + k + 1) * 128]
                            nc.tensor.matmul(sl, lt, rhs,
                                             start=False, stop=(k == KD - 1),
                                             skip_group_check=True)
                    g = dgp.tile([128, 4 * GW], FP16, tag="ggd", name="ggd")
                    nc.scalar.activation(g[:, 0:3 * GW], ps[:, 0:3 * GW],
                                         AF.Sigmoid, scale=desc_s[:, 0:1])
                    nc.scalar.activation(g[:, 3 * GW:4 * GW], ps[:, 3 * GW:4 * GW],
                                         AF.Tanh, scale=desc_s[:, 0:1])
                    t1 = dgp.tile([128, GW], FP16, tag="t1d", name="t1d")
                    nc.vector.tensor_tensor(t1[:], g[:, GW:2 * GW], c_d[:], ALU.mult)
                    t2 = dgp.tile([128, GW], FP16, tag="t2d", name="t2d")
                    nc.vector.tensor_tensor(t2[:], g[:, 0:GW], g[:, 3 * GW:4 * GW],
                                            ALU.mult)
                    nc.vector.tensor_tensor(c_d[:], t1[:], t2[:], ALU.add)
                    tc_t = dgp.tile([128, GW], FP16, tag="tcd", name="tcd")
                    nc.scalar.activation(tc_t[:], c_d[:], AF.Tanh)
                    nc.vector.scalar_tensor_tensor(hnxt[:], g[:, 2 * GW:3 * GW],
                                                   SH, tc_t[:], ALU.mult, ALU.mult)
                    hb = dgp.tile([128, GW], BF16, tag="hbd", name="hbd")
                    nc.gpsimd.tensor_tensor(hb[:], g[:, 2 * GW:3 * GW], tc_t[:],
                                            ALU.mult)
                    nc.gpsimd.tensor_copy(
                        H_T[:, :, :, step],
                        hb[:].rearrange("p (k b) -> p k b", k=KD))

            if phases < 5:
                raise _Stop
            # ---------- phase 5: attention + FFN ----------
            ctx_blk = big.tile([128, KD, BL * T], BF16, tag="ctx", name="ctx")
            mid_T = big.tile([128, KD, BL * T], BF16, tag="mid_T", name="mid_T")
            with tc.tile_pool(name="at_ps", bufs=2, space="PSUM") as aps, \
                 tc.tile_pool(name="pt_ps", bufs=2, space="PSUM") as pps, \
                 tc.tile_pool(name="cx_ps", bufs=2, space="PSUM") as cps2, \
                 tc.tile_pool(name="md_ps", bufs=2, space="PSUM") as mps, \
                 tc.tile_pool(name="at_sb", bufs=3) as asb:
                for b in range(BL):
                    a_ps = aps.tile([T, S], F32, tag="a", name="a")
                    for kd in range(2 * KE):
                        nc.tensor.matmul(a_ps[:], H_T[:, kd, b, :],
                                         mem_T[:, kd, b, :],
                                         start=(kd == 0), stop=(kd == 2 * KE - 1))
                    mx = asb.tile([T, 1], F32, tag="mx", name="mx")
                    nc.vector.tensor_reduce(mx[:], a_ps[:], AX.X, ALU.max,
                                            negate=True)
                    mx2 = asb.tile([T, 1], F32, tag="mx2", name="mx2")
                    nc.scalar.mul(mx2[:], mx[:], SCALE)
                    ex = asb.tile([T, S], F32, tag="ex", name="ex")
                    den = asb.tile([T, 1], F32, tag="den", name="den")
                    nc.scalar.activation(ex[:], a_ps[:], AF.Exp, bias=mx2[:],
                                         scale=SCALE, accum_out=den[:])
                    rec = asb.tile([T, 1], F32, tag="rec", name="rec")
                    nc.vector.reciprocal(rec[:], den[:])
                    p_sb = asb.tile([T, S], BF16, tag="p", name="p")
                    nc.vector.tensor_scalar_mul(p_sb[:], ex[:], rec[:])
                    pt_ps = pps.tile([S, T], BF16, tag="pt", name="pt",
                                     padded_shape=[128, 128])
                    nc.tensor.transpose(pt_ps[:], p_sb[:], ident[0:T, 0:T])
                    pt_sb = asb.tile([S, T], BF16, tag="pts", name="pts")
                    nc.scalar.copy(pt_sb[:], pt_ps[:])
                    for kd in range(KD):
                        c_ps = cps2.tile([128, T], F32, tag="c", name="c")
                        nc.tensor.matmul(c_ps[:], memN[:, b, kd, :], pt_sb[:],
                                         start=True, stop=True)
                        nc.vector.tensor_copy(
                            ctx_blk[:, kd, b * T:(b + 1) * T], c_ps[:])
                # FFN: mid = lrelu([H; ctx] @ W_f1.T + b1)
                for m in range(KD):
                    ps = mps.tile([128, BL * T], F32, tag="md", name="md")
                    for k in range(KD):
                        lt = wf1_s[:, (m * 8 + k) * 128:(m * 8 + k + 1) * 128]
                        nc.tensor.matmul(ps[:], lt, H_T[:, k], start=(k == 0),
                                         stop=False)
                    for k in range(KD):
                        lt = wf1_s[:, (m * 8 + KD + k) * 128:(m * 8 + KD + k + 1) * 128]
                        nc.tensor.matmul(ps[:], lt, ctx_blk[:, k, :],
                                         start=False, stop=(k == KD - 1))
                    ab = asb.tile([128, BL * T], F32, tag="mab", name="mab")
                    nc.scalar.activation(ab[:], ps[:], AF.Abs,
                                         bias=b1a_s[:, m:m + 1])
                    idt = asb.tile([128, BL * T], F32, tag="mid", name="mid")
                    nc.scalar.activation(idt[:], ps[:], AF.Identity, scale=0.55,
                                         bias=b1h_s[:, m:m + 1])
                    nc.vector.scalar_tensor_tensor(mid_T[:, m, :], ab[:], 0.45,
                                                   idt[:], ALU.mult, ALU.add)

            if phases < 6:
                raise _Stop
            # ---------- phase 6: vocab projection ----------
            wf2_3d = wf2.ap().rearrange("p (k v) -> p k v", k=KD)
            NWC = VQ // WC  # wf2 chunks per v-quarter
            SUB = 500
            NSUB = WC // SUB
            with tc.tile_pool(name="lg_ps", bufs=8, space="PSUM") as lps, \
                 tc.tile_pool(name="wf2_sb", bufs=2) as wfp, \
                 tc.tile_pool(name="lg_sb", bufs=2) as lgp:
                for vq in range(V // VQ):
                    lg = [lgp.tile([128, VQ], BF16, tag=f"lg{b}", name=f"lg{b}")
                          for b in range(BL)]
                    for wc in range(NWC):
                        v0 = vq * VQ + wc * WC
                        wf2c = wfp.tile([128, KD, WC], BF16, tag="wf2c",
                                        name="wf2c")
                        nc.sync.dma_start(wf2c[:], wf2_3d[:, :, v0:v0 + WC])
                        for b in range(BL):
                            pss = []
                            for sub in range(NSUB):
                                pss.append(lps.tile([128, SUB], F32, tag="lg",
                                                    name="lg"))
                            for k in range(KD):
                                lt = mid_T[:, k, b * T:(b + 1) * T]
                                for sub in range(NSUB):
                                    nc.tensor.matmul(
                                        pss[sub][:], lt,
                                        wf2c[:, k, sub * SUB:(sub + 1) * SUB],
                                        start=(k == 0), stop=(k == KD - 1))
                            for sub in range(NSUB):
                                dst = lg[b][:, wc * WC + sub * SUB:
                                            wc * WC + (sub + 1) * SUB]
                                nc.vector.tensor_copy(dst, pss[sub][:])
                    for b in range(BL):
                        nc.scalar.dma_start(
                            out.ap()[b, :, vq * VQ:(vq + 1) * VQ], lg[b][:])

        except _Stop:
            pass
    nc.compile()
    return nc


def prep_inputs(i, use_fp8=True, fp16_h=True):
    """Host-side staging -> list of 8 per-core in_maps."""
    def as_np(x, dt=np.float32):
        return np.ascontiguousarray(np.asarray(x), dtype=dt)

    wdt = fp8 if use_fp8 else bf16
    hdt = fp16 if fp16_h else wdt

    tok = as_np(i["tok_emb"])                  # [V, 128] f32
    inp = as_np(i["inp"], np.int64)            # [B, S]
    x = as_np(i["x"], np.int64)                # [B, T]
    start = as_np(i["start_emb"])[0]           # [128]
    est = as_np(i["enc_style_emb"])            # [2, 512]
    sty = as_np(i["style_emb"])                # [2, 512]
    lab_i = as_np(i["label_i"], np.int64)
    lab = as_np(i["label"], np.int64)

    # ---- weight scale: sw (power of 2), sh fixed ----
    absmax = max(float(np.abs(as_np(i["Whh_d"])).max()),
                 float(np.abs(as_np(i["Whh_f"])).max()),
                 float(np.abs(as_np(i["Whh_b"])).max()), 1e-30)
    sw = 32.0
    while sw * absmax > 224.0 and sw > 1.0 / 65536.0:
        sw /= 2.0
    if not use_fp8:
        sw = 1.0
    descale = np.full((128, 1), 1.0 / (sw * SH), np.float32)
    idscale = np.full((128, 1), sw * SH, np.float32)

    # ---- gate reorder [i, f, o, g] (torch order is i, f, g, o) ----
    def reorder_gates(w, H):
        # w [4H, X]: rows [i(H), f(H), g(H), o(H)] -> [i, f, o, g]
        return np.concatenate([w[0:H], w[H:2 * H], w[3 * H:4 * H], w[2 * H:3 * H]],
                              axis=0)

    def reorder_bias(v, H):
        return np.concatenate([v[0:H], v[H:2 * H], v[3 * H:4 * H], v[2 * H:3 * H]])

    # ---- decoder weights ----
    Wr = reorder_gates(as_np(i["Wih_d"]), D_DEC)          # [2048, 128]
    wih_d = np.ascontiguousarray(
        Wr.reshape(MD, 128, 128).transpose(0, 2, 1)       # (m, p_in, c)
        .transpose(1, 0, 2).reshape(128, MD * 128)).astype(bf16)
    Wr = reorder_gates(as_np(i["Whh_d"]), D_DEC) * sw     # [2048, 512]
    a = Wr.reshape(MD, 128, KD, 128)                      # [m, c, k, p]
    whh_d = np.ascontiguousarray(
        a.transpose(3, 0, 2, 1).reshape(128, MD * KD * 128)).astype(wdt)

    # ---- encoder weights (dirs fused): slab q = gate*4 + dir*2 + j ----
    def enc_tiles(Wf, Wb, ncols, dt, scale=1.0):
        # W* [1024, ncols] (torch gate order); returns [128, QE*(ncols/128)*128]
        Wf = reorder_gates(as_np(Wf), D_ENC) * scale      # [1024, ncols]
        Wb = reorder_gates(as_np(Wb), D_ENC) * scale
        nk = ncols // 128
        tiles = []
        for gate in range(4):
            for d in range(2):
                W = Wf if d == 0 else Wb
                for j in range(KE):
                    rows = W[gate * D_ENC + j * 128: gate * D_ENC + (j + 1) * 128]
                    for k in range(nk):
                        tiles.append(rows[:, k * 128:(k + 1) * 128].T)  # [p, c]
        out_ = np.stack(tiles, axis=1).reshape(128, -1)
        return np.ascontiguousarray(out_).astype(dt)

    wih_e = enc_tiles(i["Wih_f"], i["Wih_b"], 128, bf16)
    whh_e = enc_tiles(i["Whh_f"], i["Whh_b"], D_ENC, wdt, scale=sw)

    # ---- W_tr tiles (m dec-chunk, kd enc-concat chunk; kd-inner) ----
    wtr_w = as_np(i["W_tr"])                               # [512, 512]
    a = wtr_w.reshape(KD, 128, KD, 128)                    # [m, c, kd, p]
    wtr = np.ascontiguousarray(
        a.transpose(3, 0, 2, 1).reshape(128, KD * KD * 128)).astype(fp16)

    # ---- W_f1 tiles (m, k) k-inner; k 0..3 h-part, 4..7 ctx ----
    wf1_w = as_np(i["W_f1"])                               # [512, 1536? 512x(1024+512)?]
    # W_f1 [D_DEC, 2*D_ENC + D_DEC] = [512, 1024]; order [h? ...]
    # reference: ffn = concat([h, ctx]) @ W_f1.T -> cols 0:512 h, 512:1024 ctx
    a = wf1_w.reshape(KD, 128, 8, 128)                     # [m, c, k, p]
    wf1 = np.ascontiguousarray(
        a.transpose(3, 0, 2, 1).reshape(128, 4 * 8 * 128)).astype(bf16)

    # ---- W_f2 [V, 512] -> [128, KD*V] (k-major) ----
    wf2_w = as_np(i["W_f2"])                               # [V, 512]
    a = wf2_w.reshape(V, KD, 128)                          # [v, k, p]
    wf2 = np.ascontiguousarray(a.transpose(2, 1, 0).reshape(128, KD * V)
                               ).astype(bf16)

    # ---- biases ----
    def bias_cols(v, H, nm):
        vr = reorder_bias(v, H)
        return np.ascontiguousarray(vr.reshape(nm, 128).T)  # [128, nm]

    bs_d = bias_cols(as_np(i["bih_d"]) + as_np(i["bhh_d"]), D_DEC, MD)
    # encoder fused: slab q = gate*4 + dir*2 + j
    be_f = reorder_bias(as_np(i["bih_f"]) + as_np(i["bhh_f"]), D_ENC).reshape(4, KE, 128)
    be_b = reorder_bias(as_np(i["bih_b"]) + as_np(i["bhh_b"]), D_ENC).reshape(4, KE, 128)
    bs_e = np.zeros((128, QE), np.float32)
    for gate in range(4):
        for d in range(2):
            for j in range(KE):
                q = gate * 4 + d * 2 + j
                bs_e[:, q] = (be_f if d == 0 else be_b)[gate, j]
    b1 = as_np(i["b_f1"])
    b1a = np.ascontiguousarray(b1.reshape(KD, 128).T)
    b1h = np.ascontiguousarray((0.55 * b1).reshape(KD, 128).T)

    in_maps = []
    for c in range(N_CORES):
        bs = slice(c * BL, (c + 1) * BL)
        # encoder token embeddings: [128, s*BL+b]
        e = tok[inp[bs]]                                   # [BL, S, 128]
        embT_e = np.ascontiguousarray(
            e.transpose(2, 1, 0).reshape(128, S * BL)).astype(bf16)
        # decoder teacher inputs: start, x[:, :-1]
        d = np.empty((BL, T, D_EMB), np.float32)
        d[:, 0, :] = start
        d[:, 1:, :] = tok[x[bs, :T - 1]]
        embT_d = np.ascontiguousarray(
            d.transpose(2, 1, 0).reshape(128, T * BL)).astype(bf16)
        # encoder init h: [128, dir*8 + k*4 + b] scaled
        h0 = est[lab_i[bs]].reshape(BL, 2, KE, 128)        # [b, dir, k, p]
        h0e = np.ascontiguousarray(
            (h0.transpose(3, 1, 2, 0) * SH).reshape(128, 2 * KE * BL)).astype(hdt)
        # decoder init h: [128, k*4+b] scaled
        ht = sty[lab[bs]].reshape(BL, KD, 128)
        ht0 = np.ascontiguousarray(
            (ht.transpose(2, 1, 0) * SH).reshape(128, KD * BL)).astype(hdt)
        in_maps.append(dict(
            embT_e=embT_e, embT_d=embT_d, h0e_q=h0e, ht0_q=ht0,
            wih_e=wih_e, wih_d=wih_d, whh_e=whh_e, whh_d=whh_d,
            wtr=wtr, wf1=wf1, wf2=wf2,
            bs_e=bs_e, bs_d=bs_d, b1a=b1a, b1h=b1h,
            descale=descale, idscale=idscale))
    return in_maps


_NC_CACHE = {}


def kernel(**inputs):
    bias_free = all(
        not np.any(np.asarray(inputs[k]))
        for k in ("bih_f", "bhh_f", "bih_b", "bhh_b", "bih_d", "bhh_d"))
    key = f"full{bias_free}"
    if key not in _NC_CACHE:
        _NC_CACHE[key] = build(bias_free=bias_free)
    nc = _NC_CACHE[key]
    in_maps = prep_inputs(inputs)
    res = run_bass_kernel_spmd(nc, in_maps, core_ids=list(range(N_CORES)))
    return np.concatenate(
        [r["out"].astype(np.float32) for r in res.results], axis=0)


# revision 39
# speedup vs baseline: 1.0109x; 1.0109x over previous
"""DenoiseLSTM Trainium2 kernel (8 NeuronCores, SPMD, batch-parallel).

Strategy: fully data-parallel over batch — each core runs the ENTIRE network
(bi-LSTM encoder, LSTM decoder with attention, FFN, vocab projection) for its
4 of the 32 batches. No collectives; the host concatenates batch shards.

Key optimizations over the vocab-sharded baseline:
- All elementwise/softmax/FFN work shrinks 8x per core (B_local=4), so the
  LSTM steps' serial DVE/ACT chain is ~2.6x shorter per step.
- Recurrence weights quantized to fp8e4 (x sw=32); hidden state kept in
  fp16 (x sh=128) — mixed fp8-lhsT x fp16-rhs matmuls are HW-verified.
  fp8 weights halve the per-step LDWEIGHTS stream on hardware (FWL reads
  4 fp8/cycle), which is the decoder's throughput floor and is NOT modeled
  by the CoreSim cost model. Descale (1/(sw*sh)) is folded into the gate
  activations' `scale` operand.
- The x-projection contribution enters the gate PSUM through one extra wide
  matmul per step (lhsT = (sw*sh)*Identity, rhs = the step's whole xproj
  block), removing both the DVE add and 15 weight reloads from the step.
  PSUM zero-region semantics: only the first matmul per gate tile carries
  start=True, and gate PSUM tiles are padded to a full 2KB bank.
- Gate layout reordered to [i, f, o, g] so one sigmoid covers i,f,o and the
  tanh(g) tail is minimal; cell ops run on fp16 SBUF tiles (4x DVE mode);
  off-critical stores (h->memory/H) run on the otherwise idle GPSIMD
  (SBUF-only: GPSIMD cannot access PSUM).
- Token/style embedding gathers are host-side indexing; no device gathers.
- Vocab projection streams the full W_f2 [32000,512] once (bf16) while
  accumulating bf16 logits in SBUF (PE ~99% busy at the N-column roofline);
  wf2-in DMAs ride the sync queue, logits-out the scalar queue; output is
  bf16 on device and upcast to f32 on host.
"""
import sys

sys.path.insert(0, "/opt/trn_rl_repo")

from contextlib import ExitStack

import numpy as np
import ml_dtypes

import concourse.bass as bass
import concourse.bacc as bacc
import concourse.mybir as mybir
import concourse.tile as tile
from concourse.bass_utils import run_bass_kernel_spmd
from concourse.masks import make_identity

bf16 = ml_dtypes.bfloat16
fp16 = np.float16
fp8 = ml_dtypes.float8_e4m3
F32 = mybir.dt.float32
BF16 = mybir.dt.bfloat16
FP16 = mybir.dt.float16
F8 = mybir.dt.float8e4
AF = mybir.ActivationFunctionType
ALU = mybir.AluOpType
AX = mybir.AxisListType

B = 32
BL = 4                 # batches per core
D_EMB = 128
D_ENC = 256
D_DEC = 512
N_CORES = 8
S = 128
T = 128
V = 32000

KD = 4                 # decoder hidden chunks (512/128)
MD = 16                # decoder gate chunks (2048/128)
KE = 2                 # encoder hidden chunks per direction
QE = 16                # encoder fused gate slabs: gate(4) x dir(2) x j(2)
SH = 128.0             # hidden-state fp8 scale (compile-time)

VQ = 4000              # vocab output block
WC = 1000              # wf2 streaming chunk (v columns)


class _Stop(Exception):
    pass


def build(phases=7, use_fp8=True, fp16_h=True, bias_free=True):
    nc = bacc.Bacc("TRN2", target_bir_lowering=False, debug=False)
    WDT = F8 if use_fp8 else BF16
    HDT = FP16 if fp16_h else WDT
    SCALE = 1.0 / float(np.sqrt(np.float32(2 * D_ENC)))

    # ---- external inputs (host-prepped, per core) ----
    embT_e = nc.dram_tensor("embT_e", [128, S * BL], BF16, kind="ExternalInput")
    embT_d = nc.dram_tensor("embT_d", [128, T * BL], BF16, kind="ExternalInput")
    h0e_q = nc.dram_tensor("h0e_q", [128, 2 * KE * BL], HDT, kind="ExternalInput")
    ht0_q = nc.dram_tensor("ht0_q", [128, KD * BL], HDT, kind="ExternalInput")
    wih_e = nc.dram_tensor("wih_e", [128, QE * 128], BF16, kind="ExternalInput")
    wih_d = nc.dram_tensor("wih_d", [128, MD * 128], BF16, kind="ExternalInput")
    whh_e = nc.dram_tensor("whh_e", [128, QE * KE * 128], WDT, kind="ExternalInput")
    whh_d = nc.dram_tensor("whh_d", [128, MD * KD * 128], WDT, kind="ExternalInput")
    wtr = nc.dram_tensor("wtr", [128, KD * KD * 128], FP16, kind="ExternalInput")
    wf1 = nc.dram_tensor("wf1", [128, 4 * 8 * 128], BF16, kind="ExternalInput")
    wf2 = nc.dram_tensor("wf2", [128, KD * V], BF16, kind="ExternalInput")
    bs_e = nc.dram_tensor("bs_e", [128, QE], F32, kind="ExternalInput")
    bs_d = nc.dram_tensor("bs_d", [128, MD], F32, kind="ExternalInput")
    b1a = nc.dram_tensor("b1a", [128, KD], F32, kind="ExternalInput")
    b1h = nc.dram_tensor("b1h", [128, KD], F32, kind="ExternalInput")
    descale = nc.dram_tensor("descale", [128, 1], F32, kind="ExternalInput")
    idscale = nc.dram_tensor("idscale", [128, 1], F32, kind="ExternalInput")

    out = nc.dram_tensor("out", [BL, T, V], BF16, kind="ExternalOutput")

    with tile.TileContext(nc) as tc, ExitStack() as ctx:
        wpool = ctx.enter_context(tc.tile_pool(name="weights", bufs=1))
        spool = ctx.enter_context(tc.tile_pool(name="state", bufs=1))
        big = ctx.enter_context(tc.tile_pool(name="big", bufs=1))

        def load(dram, shape, dtype, tag):
            t = wpool.tile(shape, dtype, tag=tag, name=tag)
            nc.sync.dma_start(t[:], dram[:, :])
            return t

        embT_e_s = load(embT_e, [128, S * BL], BF16, "embT_e")
        embT_d_s = load(embT_d, [128, T * BL], BF16, "embT_d")
        wih_e_s = load(wih_e, [128, QE * 128], BF16, "wih_e")
        wih_d_s = load(wih_d, [128, MD * 128], BF16, "wih_d")
        whh_e_s = load(whh_e, [128, QE * KE * 128], WDT, "whh_e")
        whh_d_s = load(whh_d, [128, MD * KD * 128], WDT, "whh_d")
        wtr_s = load(wtr, [128, KD * KD * 128], FP16, "wtr")
        wf1_s = load(wf1, [128, 4 * 8 * 128], BF16, "wf1")
        bs_e_s = load(bs_e, [128, QE], F32, "bs_e")
        bs_d_s = load(bs_d, [128, MD], F32, "bs_d")
        b1a_s = load(b1a, [128, KD], F32, "b1a")
        b1h_s = load(b1h, [128, KD], F32, "b1h")
        desc_s = load(descale, [128, 1], F32, "descale")
        idsc_s = load(idscale, [128, 1], F32, "idscale")

        ident = wpool.tile([128, 128], BF16, tag="ident", name="ident")
        make_identity(nc, ident)
        # identity scaled by sw*sh: x-projection enters PSUM pre-scaled so the
        # gate activation descale (1/(sw*sh)) recovers the true pre-activation
        id_q = wpool.tile([128, 128], BF16, tag="id_q", name="id_q")
        nc.scalar.activation(id_q[:], ident[:], AF.Copy, scale=idsc_s[:, 0:1])

        # ---- state tiles ----
        h_e = [spool.tile([128, 2 * KE * BL], HDT, tag=f"h_e{j}", name=f"h_e{j}")
               for j in range(2)]
        c_e = spool.tile([128, 2 * KE * BL], FP16, tag="c_e", name="c_e")
        h_d = [spool.tile([128, KD * BL], HDT, tag=f"h_d{j}", name=f"h_d{j}")
               for j in range(2)]
        c_d = spool.tile([128, KD * BL], FP16, tag="c_d", name="c_d")
        nc.sync.dma_start(h_e[0][:], h0e_q[:, :])
        nc.sync.dma_start(h_d[0][:], ht0_q[:, :])
        nc.vector.memset(c_e[:], 0.0)

        # xproj: [128, s, q, b] so a step's rhs block is [128, q, b]
        xp_e = big.tile([128, S, QE, BL], BF16, tag="xp_e", name="xp_e")
        xp_d = big.tile([128, T, MD, BL], BF16, tag="xp_d", name="xp_d")
        # memory (encoder h) and decoder h, [128, kd, b, s/t]
        mem_T = big.tile([128, 2 * KE, BL, S], BF16, tag="mem_T", name="mem_T")
        H_T = big.tile([128, KD, BL, T], BF16, tag="H_T", name="H_T")

        # ---------- phase 1: encoder x-projection ----------
        # (decoder x-projection is emitted after the encoder loop so the
        # scheduler can run it inside encoder PE/DVE gaps)
        with tc.tile_pool(name="xp_ps", bufs=4, space="PSUM") as xps:
            def proj(w_s, src, n_q, bias_s, dst):
                for q in range(n_q):
                    ps = xps.tile([128, 512], F32, tag="xp", name="xp")
                    nc.tensor.matmul(ps[:], w_s[:, q * 128:(q + 1) * 128],
                                     src[:], start=True, stop=True)
                    ps3 = ps[:].rearrange("p (s b) -> p s b", b=BL)
                    if bias_free and q % 2 == 0:
                        nc.scalar.copy(dst[:, :, q, :], ps3)
                    elif bias_free:
                        nc.vector.tensor_copy(dst[:, :, q, :], ps3)
                    else:
                        nc.vector.tensor_scalar_add(dst[:, :, q, :], ps3,
                                                    bias_s[:, q:q + 1])

            proj(wih_e_s, embT_e_s, QE, bs_e_s, xp_e)

        try:
            if phases < 2:
                raise _Stop
            # ---------- phase 2: encoder recurrence (dirs fused) ----------
            # slab q = gate*4 + dir*2 + j;  gates ordered [i, f, o, g]
            # gate tile cols: i [0:16], f [16:32], o [32:48], g [48:64]
            # h/c cols: dir*8 + k*4 + b
            GW = 16
            with tc.tile_pool(name="enc_ps", bufs=2, space="PSUM") as eps, \
                 tc.tile_pool(name="enc_g", bufs=2) as egp:
                for step in range(S):
                    hcur = h_e[step % 2]
                    hnxt = h_e[(step + 1) % 2]
                    # full 2KB bank per gate tile: a start=True marks the
                    # whole PSUM zero-region, so tiles must not share banks
                    psf = eps.tile([128, 512], F32, tag="g", name="g")
                    ps = psf[:, 0:4 * GW]
                    # x-side: one matmul per (gate, dir) — contiguous 8-col
                    # out slabs; the first start=True marks the zero-region
                    for gv in range(4):
                        for dirv in range(2):
                            s_idx = step if dirv == 0 else S - 1 - step
                            q0 = gv * 4 + dirv * 2
                            nc.tensor.matmul(
                                ps[:, q0 * BL:(q0 + 2) * BL], id_q[:],
                                xp_e[:, s_idx, q0:q0 + 2, :],
                                start=(gv == 0 and dirv == 0), stop=False,
                                skip_group_check=True)
                    for q in range(QE):
                        d = (q >> 1) & 1
                        sl = ps[:, q * BL:(q + 1) * BL]
                        for k in range(KE):
                            rhs = hcur[:, (d * KE + k) * BL:(d * KE + k + 1) * BL]
                            lt = whh_e_s[:, (q * KE + k) * 128:(q * KE + k + 1) * 128]
                            nc.tensor.matmul(sl, lt, rhs,
                                             start=False, stop=(k == KE - 1),
                                             skip_group_check=True)
                    # gate tile f32: the i*tanh(g) = 2*(i*sig2g) - i identity
                    # cancels near 0.5, so intermediates must be f32
                    g = egp.tile([128, 4 * GW], F32, tag="gg", name="gg")
                    nc.scalar.activation(g[:], ps[:], AF.Sigmoid,
                                         scale=desc_s[:, 0:1])
                    t1 = egp.tile([128, GW], F32, tag="t1", name="t1")
                    nc.vector.tensor_tensor(t1[:], g[:, GW:2 * GW], c_e[:], ALU.mult)
                    u = egp.tile([128, GW], F32, tag="u", name="u")
                    nc.vector.tensor_tensor(u[:], g[:, 0:GW], g[:, 3 * GW:4 * GW],
                                            ALU.mult)
                    t2 = egp.tile([128, GW], F32, tag="t2", name="t2")
                    nc.vector.scalar_tensor_tensor(t2[:], u[:], 2.0, g[:, 0:GW],
                                                   ALU.mult, ALU.subtract)
                    nc.vector.tensor_tensor(c_e[:], t1[:], t2[:], ALU.add)
                    tc_t = egp.tile([128, GW], FP16, tag="tc", name="tc")
                    nc.scalar.activation(tc_t[:], c_e[:], AF.Tanh)
                    nc.vector.scalar_tensor_tensor(hnxt[:], g[:, 2 * GW:3 * GW],
                                                   SH, tc_t[:], ALU.mult, ALU.mult)
                    hb = egp.tile([128, GW], BF16, tag="hb", name="hb")
                    nc.gpsimd.tensor_tensor(hb[:], g[:, 2 * GW:3 * GW], tc_t[:],
                                            ALU.mult)
                    nc.gpsimd.tensor_copy(
                        mem_T[:, 0:KE, :, step],
                        hb[:, 0:KE * BL].rearrange("p (k b) -> p k b", k=KE))
                    nc.gpsimd.tensor_copy(
                        mem_T[:, KE:2 * KE, :, S - 1 - step],
                        hb[:, KE * BL:2 * KE * BL].rearrange("p (k b) -> p k b", k=KE))

            # decoder x-projection (independent of the encoder: fills gaps)
            with tc.tile_pool(name="xpd_ps", bufs=4, space="PSUM") as xps2:
                for q in range(MD):
                    ps = xps2.tile([128, 512], F32, tag="xpd", name="xpd")
                    nc.tensor.matmul(ps[:], wih_d_s[:, q * 128:(q + 1) * 128],
                                     embT_d_s[:], start=True, stop=True)
                    ps3 = ps[:].rearrange("p (s b) -> p s b", b=BL)
                    if bias_free and q % 2 == 0:
                        nc.scalar.copy(xp_d[:, :, q, :], ps3)
                    elif bias_free:
                        nc.vector.tensor_copy(xp_d[:, :, q, :], ps3)
                    else:
                        nc.vector.tensor_scalar_add(xp_d[:, :, q, :], ps3,
                                                    bs_d_s[:, q:q + 1])

            if phases < 3:
                raise _Stop
            # ---------- phase 3: c_t init (W_tr @ [cf;cb], lrelu) ----------
            with tc.tile_pool(name="ct_ps", bufs=2, space="PSUM") as cps, \
                 tc.tile_pool(name="ct_sb", bufs=2) as csb:
                for m in range(KD):
                    ps = cps.tile([128, BL], F32, tag="ct", name="ct")
                    for k in range(KD):
                        lt = wtr_s[:, (m * KD + k) * 128:(m * KD + k + 1) * 128]
                        nc.tensor.matmul(ps[:], lt, c_e[:, k * BL:(k + 1) * BL],
                                         start=(k == 0), stop=(k == KD - 1))
                    ab = csb.tile([128, BL], F32, tag="ab", name="ab")
                    nc.scalar.activation(ab[:], ps[:], AF.Abs)
                    idt = csb.tile([128, BL], F32, tag="idt", name="idt")
                    nc.scalar.activation(idt[:], ps[:], AF.Identity, scale=0.55)
                    nc.vector.scalar_tensor_tensor(c_d[:, m * BL:(m + 1) * BL],
                                                   ab[:], 0.45, idt[:],
                                                   ALU.mult, ALU.add)

            if phases < 4:
                raise _Stop
            # mem_norm transposes for attention (only need mem_T): spread
            # through the decoder loop to fill PE/ACT gaps
            memN = big.tile([S, BL, 2 * KE, 128], BF16, tag="memN", name="memN")

            # ---------- phase 4: decoder recurrence ----------
            # slab m = gate*4 + j; gates [i, f, o, g]; h/c cols k*4+b
            GW = KD * BL  # 16
            with tc.tile_pool(name="dec_ps", bufs=2, space="PSUM") as dps, \
                 tc.tile_pool(name="dec_g", bufs=2) as dgp, \
                 tc.tile_pool(name="mn_ps", bufs=2, space="PSUM") as mnpool:
                for step in range(T):
                    if step % 8 == 4 and step // 8 < 2 * KE * BL:
                        idx = step // 8
                        b, kd = idx // (2 * KE), idx % (2 * KE)
                        mn_ps = mnpool.tile([S, 128], BF16, tag="mn", name="mn",
                                            padded_shape=[128, 128])
                        nc.tensor.transpose(mn_ps[:], mem_T[:, kd, b, :],
                                            ident[:])
                        nc.scalar.copy(memN[:, b, kd, :], mn_ps[:])
                    hcur = h_d[step % 2]
                    hnxt = h_d[(step + 1) % 2]
                    psf = dps.tile([128, 512], F32, tag="gd", name="gd")
                    ps = psf[:, 0:4 * GW]
                    # x-side: single wide matmul (rhs = whole step's xproj
                    # block); start=True marks the tile's zero-region once
                    nc.tensor.matmul(ps[:], id_q[:], xp_d[:, step, :, :],
                                     start=True, stop=False,
                                     skip_group_check=True)
                    for m in range(MD):
                        sl = ps[:, m * BL:(m + 1) * BL]
                        for k in range(KD):
                            rhs = hcur[:, k * BL:(k + 1) * BL]
                            lt = whh_d_s[:, (m * KD + k) * 128:
                                         (m * KD + k + 1) * 128]
                            nc.tensor.matmul(sl, lt, rhs,
                                             start=False, stop=(k == KD - 1),
                                             skip_group_check=True)
                    g = dgp.tile([128, 4 * GW], F32, tag="ggd", name="ggd")
                    nc.scalar.activation(g[:], ps[:], AF.Sigmoid,
                                         scale=desc_s[:, 0:1])
                    t1 = dgp.tile([128, GW], F32, tag="t1d", name="t1d")
                    nc.vector.tensor_tensor(t1[:], g[:, GW:2 * GW], c_d[:], ALU.mult)
                    u = dgp.tile([128, GW], F32, tag="ud", name="ud")
                    nc.vector.tensor_tensor(u[:], g[:, 0:GW], g[:, 3 * GW:4 * GW],
                                            ALU.mult)
                    t2 = dgp.tile([128, GW], F32, tag="t2d", name="t2d")
                    nc.vector.scalar_tensor_tensor(t2[:], u[:], 2.0, g[:, 0:GW],
                                                   ALU.mult, ALU.subtract)
                    nc.vector.tensor_tensor(c_d[:], t1[:], t2[:], ALU.add)
                    tc_t = dgp.tile([128, GW], FP16, tag="tcd", name="tcd")
                    nc.scalar.activation(tc_t[:], c_d[:], AF.Tanh)
                    nc.vector.scalar_tensor_tensor(hnxt[:], g[:, 2 * GW:3 * GW],
                                                   SH, tc_t[:], ALU.mult, ALU.mult)
                    hb = dgp.tile([128, GW], BF16, tag="hbd", name="hbd")
                    nc.gpsimd.tensor_tensor(hb[:], g[:, 2 * GW:3 * GW], tc_t[:],
                                            ALU.mult)
                    nc.gpsimd.tensor_copy(
                        H_T[:, :, :, step],
                        hb[:].rearrange("p (k b) -> p k b", k=KD))

            if phases < 5:
                raise _Stop
            # ---------- phase 5: attention + FFN ----------
            ctx_blk = big.tile([128, KD, BL * T], BF16, tag="ctx", name="ctx")
            mid_T = big.tile([128, KD, BL * T], BF16, tag="mid_T", name="mid_T")
            with tc.tile_pool(name="at_ps", bufs=2, space="PSUM") as aps, \
                 tc.tile_pool(name="pt_ps", bufs=2, space="PSUM") as pps, \
                 tc.tile_pool(name="cx_ps", bufs=2, space="PSUM") as cps2, \
                 tc.tile_pool(name="md_ps", bufs=2, space="PSUM") as mps, \
                 tc.tile_pool(name="at_sb", bufs=3) as asb:
                for b in range(BL):
                    a_ps = aps.tile([T, S], F32, tag="a", name="a")
                    for kd in range(2 * KE):
                        nc.tensor.matmul(a_ps[:], H_T[:, kd, b, :],
                                         mem_T[:, kd, b, :],
                                         start=(kd == 0), stop=(kd == 2 * KE - 1))
                    mx = asb.tile([T, 1], F32, tag="mx", name="mx")
                    nc.vector.tensor_reduce(mx[:], a_ps[:], AX.X, ALU.max,
                                            negate=True)
                    mx2 = asb.tile([T, 1], F32, tag="mx2", name="mx2")
                    nc.scalar.mul(mx2[:], mx[:], SCALE)
                    ex = asb.tile([T, S], F32, tag="ex", name="ex")
                    den = asb.tile([T, 1], F32, tag="den", name="den")
                    nc.scalar.activation(ex[:], a_ps[:], AF.Exp, bias=mx2[:],
                                         scale=SCALE, accum_out=den[:])
                    rec = asb.tile([T, 1], F32, tag="rec", name="rec")
                    nc.vector.reciprocal(rec[:], den[:])
                    p_sb = asb.tile([T, S], BF16, tag="p", name="p")
                    nc.vector.tensor_scalar_mul(p_sb[:], ex[:], rec[:])
                    pt_ps = pps.tile([S, T], BF16, tag="pt", name="pt",
                                     padded_shape=[128, 128])
                    nc.tensor.transpose(pt_ps[:], p_sb[:], ident[0:T, 0:T])
                    pt_sb = asb.tile([S, T], BF16, tag="pts", name="pts")
                    nc.scalar.copy(pt_sb[:], pt_ps[:])
                    for kd in range(KD):
                        c_ps = cps2.tile([128, T], F32, tag="c", name="c")
                        nc.tensor.matmul(c_ps[:], memN[:, b, kd, :], pt_sb[:],
                                         start=True, stop=True)
                        nc.vector.tensor_copy(
                            ctx_blk[:, kd, b * T:(b + 1) * T], c_ps[:])
                # FFN: mid = lrelu([H; ctx] @ W_f1.T + b1)
                for m in range(KD):
                    ps = mps.tile([128, BL * T], F32, tag="md", name="md")
                    for k in range(KD):
                        lt = wf1_s[:, (m * 8 + k) * 128:(m * 8 + k + 1) * 128]
                        nc.tensor.matmul(ps[:], lt, H_T[:, k], start=(k == 0),
                                         stop=False)
                    for k in range(KD):
                        lt = wf1_s[:, (m * 8 + KD + k) * 128:(m * 8 + KD + k + 1) * 128]
                        nc.tensor.matmul(ps[:], lt, ctx_blk[:, k, :],
                                         start=False, stop=(k == KD - 1))
                    ab = asb.tile([128, BL * T], F32, tag="mab", name="mab")
                    nc.scalar.activation(ab[:], ps[:], AF.Abs,
                                         bias=b1a_s[:, m:m + 1])
                    idt = asb.tile([128, BL * T], F32, tag="mid", name="mid")
                    nc.scalar.activation(idt[:], ps[:], AF.Identity, scale=0.55,
                                         bias=b1h_s[:, m:m + 1])
                    nc.vector.scalar_tensor_tensor(mid_T[:, m, :], ab[:], 0.45,
                                                   idt[:], ALU.mult, ALU.add)

            if phases < 6:
                raise _Stop
            # ---------- phase 6: vocab projection ----------
            wf2_3d = wf2.ap().rearrange("p (k v) -> p k v", k=KD)
            NWC = VQ // WC  # wf2 chunks per v-quarter
            SUB = 500
            NSUB = WC // SUB
            with tc.tile_pool(name="lg_ps", bufs=8, space="PSUM") as lps, \
                 tc.tile_pool(name="wf2_sb", bufs=2) as wfp, \
                 tc.tile_pool(name="lg_sb", bufs=2) as lgp:
                for vq in range(V // VQ):
                    lg = [lgp.tile([128, VQ], BF16, tag=f"lg{b}", name=f"lg{b}")
                          for b in range(BL)]
                    for wc in range(NWC):
                        v0 = vq * VQ + wc * WC
                        wf2c = wfp.tile([128, KD, WC], BF16, tag="wf2c",
                                        name="wf2c")
                        nc.sync.dma_start(wf2c[:], wf2_3d[:, :, v0:v0 + WC])
                        for b in range(BL):
                            pss = []
                            for sub in range(NSUB):
                                pss.append(lps.tile([128, SUB], F32, tag="lg",
                                                    name="lg"))
                            for k in range(KD):
                                lt = mid_T[:, k, b * T:(b + 1) * T]
                                for sub in range(NSUB):
                                    nc.tensor.matmul(
                                        pss[sub][:], lt,
                                        wf2c[:, k, sub * SUB:(sub + 1) * SUB],
                                        start=(k == 0), stop=(k == KD - 1))
                            for sub in range(NSUB):
                                dst = lg[b][:, wc * WC + sub * SUB:
                                            wc * WC + (sub + 1) * SUB]
                                nc.vector.tensor_copy(dst, pss[sub][:])
                    for b in range(BL):
                        nc.scalar.dma_start(
                            out.ap()[b, :, vq * VQ:(vq + 1) * VQ], lg[b][:])

        except _Stop:
            pass
    nc.compile()
    return nc


def prep_inputs(i, use_fp8=True, fp16_h=True):
    """Host-side staging -> list of 8 per-core in_maps."""
    def as_np(x, dt=np.float32):
        return np.ascontiguousarray(np.asarray(x), dtype=dt)

    wdt = fp8 if use_fp8 else bf16
    hdt = fp16 if fp16_h else wdt

    tok = as_np(i["tok_emb"])                  # [V, 128] f32
    inp = as_np(i["inp"], np.int64)            # [B, S]
    x = as_np(i["x"], np.int64)                # [B, T]
    start = as_np(i["start_emb"])[0]           # [128]
    est = as_np(i["enc_style_emb"])            # [2, 512]
    sty = as_np(i["style_emb"])                # [2, 512]
    lab_i = as_np(i["label_i"], np.int64)
    lab = as_np(i["label"], np.int64)

    # ---- weight scale: sw (power of 2), sh fixed ----
    absmax = max(float(np.abs(as_np(i["Whh_d"])).max()),
                 float(np.abs(as_np(i["Whh_f"])).max()),
                 float(np.abs(as_np(i["Whh_b"])).max()), 1e-30)
    sw = 32.0
    while sw * absmax > 224.0 and sw > 1.0 / 65536.0:
        sw /= 2.0
    if not use_fp8:
        sw = 1.0
    descale = np.full((128, 1), 1.0 / (sw * SH), np.float32)
    idscale = np.full((128, 1), sw * SH, np.float32)

    # ---- gate reorder [i, f, o, g] (torch order is i, f, g, o) ----
    # g-gate rows are pre-scaled x2: the kernel computes all four gate
    # nonlinearities with ONE sigmoid op and recovers
    # tanh(g) = 2*sigmoid(2g) - 1 on the DVE (exact identity).
    def reorder_gates(w, H):
        # w [4H, X]: rows [i(H), f(H), g(H), o(H)] -> [i, f, o, 2*g]
        return np.concatenate([w[0:H], w[H:2 * H], w[3 * H:4 * H],
                               2.0 * w[2 * H:3 * H]], axis=0)

    def reorder_bias(v, H):
        return np.concatenate([v[0:H], v[H:2 * H], v[3 * H:4 * H],
                               2.0 * v[2 * H:3 * H]])

    # ---- decoder weights ----
    Wr = reorder_gates(as_np(i["Wih_d"]), D_DEC)          # [2048, 128]
    wih_d = np.ascontiguousarray(
        Wr.reshape(MD, 128, 128).transpose(0, 2, 1)       # (m, p_in, c)
        .transpose(1, 0, 2).reshape(128, MD * 128)).astype(bf16)
    Wr = reorder_gates(as_np(i["Whh_d"]), D_DEC) * sw     # [2048, 512]
    a = Wr.reshape(MD, 128, KD, 128)                      # [m, c, k, p]
    whh_d = np.ascontiguousarray(
        a.transpose(3, 0, 2, 1).reshape(128, MD * KD * 128)).astype(wdt)

    # ---- encoder weights (dirs fused): slab q = gate*4 + dir*2 + j ----
    def enc_tiles(Wf, Wb, ncols, dt, scale=1.0):
        # W* [1024, ncols] (torch gate order); returns [128, QE*(ncols/128)*128]
        Wf = reorder_gates(as_np(Wf), D_ENC) * scale      # [1024, ncols]
        Wb = reorder_gates(as_np(Wb), D_ENC) * scale
        nk = ncols // 128
        tiles = []
        for gate in range(4):
            for d in range(2):
                W = Wf if d == 0 else Wb
                for j in range(KE):
                    rows = W[gate * D_ENC + j * 128: gate * D_ENC + (j + 1) * 128]
                    for k in range(nk):
                        tiles.append(rows[:, k * 128:(k + 1) * 128].T)  # [p, c]
        out_ = np.stack(tiles, axis=1).reshape(128, -1)
        return np.ascontiguousarray(out_).astype(dt)

    wih_e = enc_tiles(i["Wih_f"], i["Wih_b"], 128, bf16)
    whh_e = enc_tiles(i["Whh_f"], i["Whh_b"], D_ENC, wdt, scale=sw)

    # ---- W_tr tiles (m dec-chunk, kd enc-concat chunk; kd-inner) ----
    wtr_w = as_np(i["W_tr"])                               # [512, 512]
    a = wtr_w.reshape(KD, 128, KD, 128)                    # [m, c, kd, p]
    wtr = np.ascontiguousarray(
        a.transpose(3, 0, 2, 1).reshape(128, KD * KD * 128)).astype(fp16)

    # ---- W_f1 tiles (m, k) k-inner; k 0..3 h-part, 4..7 ctx ----
    wf1_w = as_np(i["W_f1"])                               # [512, 1536? 512x(1024+512)?]
    # W_f1 [D_DEC, 2*D_ENC + D_DEC] = [512, 1024]; order [h? ...]
    # reference: ffn = concat([h, ctx]) @ W_f1.T -> cols 0:512 h, 512:1024 ctx
    a = wf1_w.reshape(KD, 128, 8, 128)                     # [m, c, k, p]
    wf1 = np.ascontiguousarray(
        a.transpose(3, 0, 2, 1).reshape(128, 4 * 8 * 128)).astype(bf16)

    # ---- W_f2 [V, 512] -> [128, KD*V] (k-major) ----
    wf2_w = as_np(i["W_f2"])                               # [V, 512]
    a = wf2_w.reshape(V, KD, 128)                          # [v, k, p]
    wf2 = np.ascontiguousarray(a.transpose(2, 1, 0).reshape(128, KD * V)
                               ).astype(bf16)

    # ---- biases ----
    def bias_cols(v, H, nm):
        vr = reorder_bias(v, H)
        return np.ascontiguousarray(vr.reshape(nm, 128).T)  # [128, nm]

    bs_d = bias_cols(as_np(i["bih_d"]) + as_np(i["bhh_d"]), D_DEC, MD)
    # encoder fused: slab q = gate*4 + dir*2 + j
    be_f = reorder_bias(as_np(i["bih_f"]) + as_np(i["bhh_f"]), D_ENC).reshape(4, KE, 128)
    be_b = reorder_bias(as_np(i["bih_b"]) + as_np(i["bhh_b"]), D_ENC).reshape(4, KE, 128)
    bs_e = np.zeros((128, QE), np.float32)
    for gate in range(4):
        for d in range(2):
            for j in range(KE):
                q = gate * 4 + d * 2 + j
                bs_e[:, q] = (be_f if d == 0 else be_b)[gate, j]
    b1 = as_np(i["b_f1"])
    b1a = np.ascontiguousarray(b1.reshape(KD, 128).T)
    b1h = np.ascontiguousarray((0.55 * b1).reshape(KD, 128).T)

    in_maps = []
    for c in range(N_CORES):
        bs = slice(c * BL, (c + 1) * BL)
        # encoder token embeddings: [128, s*BL+b]
        e = tok[inp[bs]]                                   # [BL, S, 128]
        embT_e = np.ascontiguousarray(
            e.transpose(2, 1, 0).reshape(128, S * BL)).astype(bf16)
        # decoder teacher inputs: start, x[:, :-1]
        d = np.empty((BL, T, D_EMB), np.float32)
        d[:, 0, :] = start
        d[:, 1:, :] = tok[x[bs, :T - 1]]
        embT_d = np.ascontiguousarray(
            d.transpose(2, 1, 0).reshape(128, T * BL)).astype(bf16)
        # encoder init h: [128, dir*8 + k*4 + b] scaled
        h0 = est[lab_i[bs]].reshape(BL, 2, KE, 128)        # [b, dir, k, p]
        h0e = np.ascontiguousarray(
            (h0.transpose(3, 1, 2, 0) * SH).reshape(128, 2 * KE * BL)).astype(hdt)
        # decoder init h: [128, k*4+b] scaled
        ht = sty[lab[bs]].reshape(BL, KD, 128)
        ht0 = np.ascontiguousarray(
            (ht.transpose(2, 1, 0) * SH).reshape(128, KD * BL)).astype(hdt)
        in_maps.append(dict(
            embT_e=embT_e, embT_d=embT_d, h0e_q=h0e, ht0_q=ht0,
            wih_e=wih_e, wih_d=wih_d, whh_e=whh_e, whh_d=whh_d,
            wtr=wtr, wf1=wf1, wf2=wf2,
            bs_e=bs_e, bs_d=bs_d, b1a=b1a, b1h=b1h,
            descale=descale, idscale=idscale))
    return in_maps


_NC_CACHE = {}


def kernel(**inputs):
    bias_free = all(
        not np.any(np.asarray(inputs[k]))
        for k in ("bih_f", "bhh_f", "bih_b", "bhh_b", "bih_d", "bhh_d"))
    key = f"full{bias_free}"
    if key not in _NC_CACHE:
        _NC_CACHE[key] = build(bias_free=bias_free)
    nc = _NC_CACHE[key]
    in_maps = prep_inputs(inputs)
    res = run_bass_kernel_spmd(nc, in_maps, core_ids=list(range(N_CORES)))
    return np.concatenate(
        [r["out"].astype(np.float32) for r in res.results], axis=0)


# revision 42
# speedup vs baseline: 1.0113x; 1.0004x over previous
"""DenoiseLSTM Trainium2 kernel (8 NeuronCores, SPMD, batch-parallel).

Strategy: fully data-parallel over batch — each core runs the ENTIRE network
(bi-LSTM encoder, LSTM decoder with attention, FFN, vocab projection) for its
4 of the 32 batches. No collectives; the host concatenates batch shards.

Key optimizations over the vocab-sharded baseline:
- All elementwise/softmax/FFN work shrinks 8x per core (B_local=4), so the
  LSTM steps' serial DVE/ACT chain is ~2.6x shorter per step.
- Recurrence weights quantized to fp8e4 (x sw=32); hidden state kept in
  fp16 (x sh=128) — mixed fp8-lhsT x fp16-rhs matmuls are HW-verified.
  fp8 weights halve the per-step LDWEIGHTS stream on hardware (FWL reads
  4 fp8/cycle), which is the decoder's throughput floor and is NOT modeled
  by the CoreSim cost model. Descale (1/(sw*sh)) is folded into the gate
  activations' `scale` operand.
- The x-projection contribution enters the gate PSUM through one extra wide
  matmul per step (lhsT = (sw*sh)*Identity, rhs = the step's whole xproj
  block), removing both the DVE add and 15 weight reloads from the step.
  PSUM zero-region semantics: only the first matmul per gate tile carries
  start=True, and gate PSUM tiles are padded to a full 2KB bank.
- ALL four gate nonlinearities run as ONE sigmoid op: g-gate weights are
  pre-scaled x2 on the host and tanh(g) = 2*sigmoid(2g) - 1 is recovered on
  the DVE (exact identity; intermediates kept f32 to avoid cancellation).
  Off-critical stores (h->memory/H) run on the otherwise idle GPSIMD
  (SBUF-only: GPSIMD cannot access PSUM).
- Token/style embedding gathers are host-side indexing; no device gathers.
- Vocab projection streams the full W_f2 [32000,512] once (bf16) while
  accumulating bf16 logits in SBUF (PE ~99% busy at the N-column roofline);
  wf2-in DMAs ride the sync queue, logits-out the scalar queue; output is
  bf16 on device and upcast to f32 on host.
"""
import sys

sys.path.insert(0, "/opt/trn_rl_repo")

from contextlib import ExitStack

import numpy as np
import ml_dtypes

import concourse.bass as bass
import concourse.bacc as bacc
import concourse.mybir as mybir
import concourse.tile as tile
from concourse.bass_utils import run_bass_kernel_spmd
from concourse.masks import make_identity

bf16 = ml_dtypes.bfloat16
fp16 = np.float16
fp8 = ml_dtypes.float8_e4m3
F32 = mybir.dt.float32
BF16 = mybir.dt.bfloat16
FP16 = mybir.dt.float16
F8 = mybir.dt.float8e4
AF = mybir.ActivationFunctionType
ALU = mybir.AluOpType
AX = mybir.AxisListType

B = 32
BL = 4                 # batches per core
D_EMB = 128
D_ENC = 256
D_DEC = 512
N_CORES = 8
S = 128
T = 128
V = 32000

KD = 4                 # decoder hidden chunks (512/128)
MD = 16                # decoder gate chunks (2048/128)
KE = 2                 # encoder hidden chunks per direction
QE = 16                # encoder fused gate slabs: gate(4) x dir(2) x j(2)
SH = 128.0             # hidden-state fp8 scale (compile-time)

VQ = 4000              # vocab output block
WC = 1000              # wf2 streaming chunk (v columns)


class _Stop(Exception):
    pass


def build(phases=7, use_fp8=True, fp16_h=True, bias_free=True):
    nc = bacc.Bacc("TRN2", target_bir_lowering=False, debug=False)
    WDT = F8 if use_fp8 else BF16
    HDT = FP16 if fp16_h else WDT
    SCALE = 1.0 / float(np.sqrt(np.float32(2 * D_ENC)))

    # ---- external inputs (host-prepped, per core) ----
    embT_e = nc.dram_tensor("embT_e", [128, S * BL], BF16, kind="ExternalInput")
    embT_d = nc.dram_tensor("embT_d", [128, T * BL], BF16, kind="ExternalInput")
    h0e_q = nc.dram_tensor("h0e_q", [128, 2 * KE * BL], HDT, kind="ExternalInput")
    ht0_q = nc.dram_tensor("ht0_q", [128, KD * BL], HDT, kind="ExternalInput")
    wih_e = nc.dram_tensor("wih_e", [128, QE * 128], BF16, kind="ExternalInput")
    wih_d = nc.dram_tensor("wih_d", [128, MD * 128], BF16, kind="ExternalInput")
    whh_e = nc.dram_tensor("whh_e", [128, QE * KE * 128], WDT, kind="ExternalInput")
    whh_d = nc.dram_tensor("whh_d", [128, MD * KD * 128], WDT, kind="ExternalInput")
    wtr = nc.dram_tensor("wtr", [128, KD * KD * 128], FP16, kind="ExternalInput")
    wf1 = nc.dram_tensor("wf1", [128, 4 * 8 * 128], BF16, kind="ExternalInput")
    wf2 = nc.dram_tensor("wf2", [128, KD * V], BF16, kind="ExternalInput")
    bs_e = nc.dram_tensor("bs_e", [128, QE], F32, kind="ExternalInput")
    bs_d = nc.dram_tensor("bs_d", [128, MD], F32, kind="ExternalInput")
    b1a = nc.dram_tensor("b1a", [128, KD], F32, kind="ExternalInput")
    b1h = nc.dram_tensor("b1h", [128, KD], F32, kind="ExternalInput")
    descale = nc.dram_tensor("descale", [128, 1], F32, kind="ExternalInput")
    idscale = nc.dram_tensor("idscale", [128, 1], F32, kind="ExternalInput")

    out = nc.dram_tensor("out", [BL, T, V], BF16, kind="ExternalOutput")

    with tile.TileContext(nc) as tc, ExitStack() as ctx:
        wpool = ctx.enter_context(tc.tile_pool(name="weights", bufs=1))
        spool = ctx.enter_context(tc.tile_pool(name="state", bufs=1))
        big = ctx.enter_context(tc.tile_pool(name="big", bufs=1))

        def load(dram, shape, dtype, tag, eng=None):
            t = wpool.tile(shape, dtype, tag=tag, name=tag)
            (eng or nc.sync).dma_start(t[:], dram[:, :])
            return t

        # encoder-critical loads on the sync queue, in need-order; the
        # decoder-side weights ride the scalar queue so ~20 dma_starts don't
        # serialize on one DGE sequencer (~565ns each)
        desc_s = load(descale, [128, 1], F32, "descale")
        idsc_s = load(idscale, [128, 1], F32, "idscale")
        embT_e_s = load(embT_e, [128, S * BL], BF16, "embT_e")
        wih_e_s = load(wih_e, [128, QE * 128], BF16, "wih_e")
        whh_e_s = load(whh_e, [128, QE * KE * 128], WDT, "whh_e")
        bs_e_s = load(bs_e, [128, QE], F32, "bs_e")
        embT_d_s = load(embT_d, [128, T * BL], BF16, "embT_d", nc.scalar)
        wih_d_s = load(wih_d, [128, MD * 128], BF16, "wih_d", nc.scalar)
        whh_d_s = load(whh_d, [128, MD * KD * 128], WDT, "whh_d", nc.scalar)
        wtr_s = load(wtr, [128, KD * KD * 128], FP16, "wtr", nc.scalar)
        wf1_s = load(wf1, [128, 4 * 8 * 128], BF16, "wf1", nc.scalar)
        bs_d_s = load(bs_d, [128, MD], F32, "bs_d", nc.scalar)
        b1a_s = load(b1a, [128, KD], F32, "b1a", nc.scalar)
        b1h_s = load(b1h, [128, KD], F32, "b1h", nc.scalar)

        ident = wpool.tile([128, 128], BF16, tag="ident", name="ident")
        make_identity(nc, ident)
        # identity scaled by sw*sh: x-projection enters PSUM pre-scaled so the
        # gate activation descale (1/(sw*sh)) recovers the true pre-activation
        id_q = wpool.tile([128, 128], BF16, tag="id_q", name="id_q")
        nc.scalar.activation(id_q[:], ident[:], AF.Copy, scale=idsc_s[:, 0:1])

        # ---- state tiles ----
        h_e = [spool.tile([128, 2 * KE * BL], HDT, tag=f"h_e{j}", name=f"h_e{j}")
               for j in range(2)]
        c_e = spool.tile([128, 2 * KE * BL], FP16, tag="c_e", name="c_e")
        h_d = [spool.tile([128, KD * BL], HDT, tag=f"h_d{j}", name=f"h_d{j}")
               for j in range(2)]
        c_d = spool.tile([128, KD * BL], FP16, tag="c_d", name="c_d")
        nc.sync.dma_start(h_e[0][:], h0e_q[:, :])
        nc.sync.dma_start(h_d[0][:], ht0_q[:, :])
        nc.vector.memset(c_e[:], 0.0)

        # xproj: [128, s, q, b] so a step's rhs block is [128, q, b]
        xp_e = big.tile([128, S, QE, BL], BF16, tag="xp_e", name="xp_e")
        xp_d = big.tile([128, T, MD, BL], BF16, tag="xp_d", name="xp_d")
        # memory (encoder h) and decoder h, [128, kd, b, s/t]
        mem_T = big.tile([128, 2 * KE, BL, S], BF16, tag="mem_T", name="mem_T")
        H_T = big.tile([128, KD, BL, T], BF16, tag="H_T", name="H_T")

        # ---------- phase 1: encoder x-projection ----------
        # (decoder x-projection is emitted after the encoder loop so the
        # scheduler can run it inside encoder PE/DVE gaps)
        with tc.tile_pool(name="xp_ps", bufs=4, space="PSUM") as xps:
            def proj(w_s, src, n_q, bias_s, dst):
                for q in range(n_q):
                    ps = xps.tile([128, 512], F32, tag="xp", name="xp")
                    nc.tensor.matmul(ps[:], w_s[:, q * 128:(q + 1) * 128],
                                     src[:], start=True, stop=True)
                    ps3 = ps[:].rearrange("p (s b) -> p s b", b=BL)
                    if bias_free and q % 2 == 0:
                        nc.scalar.copy(dst[:, :, q, :], ps3)
                    elif bias_free:
                        nc.vector.tensor_copy(dst[:, :, q, :], ps3)
                    else:
                        nc.vector.tensor_scalar_add(dst[:, :, q, :], ps3,
                                                    bias_s[:, q:q + 1])

            proj(wih_e_s, embT_e_s, QE, bs_e_s, xp_e)

        try:
            if phases < 2:
                raise _Stop
            # ---------- phase 2: encoder recurrence (dirs fused) ----------
            # slab q = gate*4 + dir*2 + j;  gates ordered [i, f, o, g]
            # gate tile cols: i [0:16], f [16:32], o [32:48], g [48:64]
            # h/c cols: dir*8 + k*4 + b
            GW = 16
            with tc.tile_pool(name="enc_ps", bufs=2, space="PSUM") as eps, \
                 tc.tile_pool(name="enc_g", bufs=2) as egp:
                for step in range(S):
                    hcur = h_e[step % 2]
                    hnxt = h_e[(step + 1) % 2]
                    # full 2KB bank per gate tile: a start=True marks the
                    # whole PSUM zero-region, so tiles must not share banks
                    psf = eps.tile([128, 512], F32, tag="g", name="g")
                    ps = psf[:, 0:4 * GW]
                    # x-side: one matmul per (gate, dir) — contiguous 8-col
                    # out slabs; the first start=True marks the zero-region
                    for gv in range(4):
                        for dirv in range(2):
                            s_idx = step if dirv == 0 else S - 1 - step
                            q0 = gv * 4 + dirv * 2
                            nc.tensor.matmul(
                                ps[:, q0 * BL:(q0 + 2) * BL], id_q[:],
                                xp_e[:, s_idx, q0:q0 + 2, :],
                                start=(gv == 0 and dirv == 0), stop=False,
                                skip_group_check=True)
                    for q in range(QE):
                        d = (q >> 1) & 1
                        sl = ps[:, q * BL:(q + 1) * BL]
                        for k in range(KE):
                            rhs = hcur[:, (d * KE + k) * BL:(d * KE + k + 1) * BL]
                            lt = whh_e_s[:, (q * KE + k) * 128:(q * KE + k + 1) * 128]
                            nc.tensor.matmul(sl, lt, rhs,
                                             start=False, stop=(k == KE - 1),
                                             skip_group_check=True)
                    # gate tile f32: the i*tanh(g) = 2*(i*sig2g) - i identity
                    # cancels near 0.5, so intermediates must be f32
                    g = egp.tile([128, 4 * GW], F32, tag="gg", name="gg")
                    nc.scalar.activation(g[:], ps[:], AF.Sigmoid,
                                         scale=desc_s[:, 0:1])
                    t1 = egp.tile([128, GW], F32, tag="t1", name="t1")
                    nc.vector.tensor_tensor(t1[:], g[:, GW:2 * GW], c_e[:], ALU.mult)
                    u = egp.tile([128, GW], F32, tag="u", name="u")
                    nc.vector.tensor_tensor(u[:], g[:, 0:GW], g[:, 3 * GW:4 * GW],
                                            ALU.mult)
                    t2 = egp.tile([128, GW], F32, tag="t2", name="t2")
                    nc.vector.scalar_tensor_tensor(t2[:], u[:], 2.0, g[:, 0:GW],
                                                   ALU.mult, ALU.subtract)
                    nc.vector.tensor_tensor(c_e[:], t1[:], t2[:], ALU.add)
                    tc_t = egp.tile([128, GW], FP16, tag="tc", name="tc")
                    nc.scalar.activation(tc_t[:], c_e[:], AF.Tanh)
                    nc.vector.scalar_tensor_tensor(hnxt[:], g[:, 2 * GW:3 * GW],
                                                   SH, tc_t[:], ALU.mult, ALU.mult)
                    hb = egp.tile([128, GW], BF16, tag="hb", name="hb")
                    nc.gpsimd.tensor_tensor(hb[:], g[:, 2 * GW:3 * GW], tc_t[:],
                                            ALU.mult)
                    nc.gpsimd.tensor_copy(
                        mem_T[:, 0:KE, :, step],
                        hb[:, 0:KE * BL].rearrange("p (k b) -> p k b", k=KE))
                    nc.gpsimd.tensor_copy(
                        mem_T[:, KE:2 * KE, :, S - 1 - step],
                        hb[:, KE * BL:2 * KE * BL].rearrange("p (k b) -> p k b", k=KE))

            # decoder x-projection (independent of the encoder: fills gaps)
            with tc.tile_pool(name="xpd_ps", bufs=4, space="PSUM") as xps2:
                for q in range(MD):
                    ps = xps2.tile([128, 512], F32, tag="xpd", name="xpd")
                    nc.tensor.matmul(ps[:], wih_d_s[:, q * 128:(q + 1) * 128],
                                     embT_d_s[:], start=True, stop=True)
                    ps3 = ps[:].rearrange("p (s b) -> p s b", b=BL)
                    if bias_free and q % 2 == 0:
                        nc.scalar.copy(xp_d[:, :, q, :], ps3)
                    elif bias_free:
                        nc.vector.tensor_copy(xp_d[:, :, q, :], ps3)
                    else:
                        nc.vector.tensor_scalar_add(xp_d[:, :, q, :], ps3,
                                                    bs_d_s[:, q:q + 1])

            if phases < 3:
                raise _Stop
            # ---------- phase 3: c_t init (W_tr @ [cf;cb], lrelu) ----------
            with tc.tile_pool(name="ct_ps", bufs=2, space="PSUM") as cps, \
                 tc.tile_pool(name="ct_sb", bufs=2) as csb:
                for m in range(KD):
                    ps = cps.tile([128, BL], F32, tag="ct", name="ct")
                    for k in range(KD):
                        lt = wtr_s[:, (m * KD + k) * 128:(m * KD + k + 1) * 128]
                        nc.tensor.matmul(ps[:], lt, c_e[:, k * BL:(k + 1) * BL],
                                         start=(k == 0), stop=(k == KD - 1))
                    ab = csb.tile([128, BL], F32, tag="ab", name="ab")
                    nc.scalar.activation(ab[:], ps[:], AF.Abs)
                    idt = csb.tile([128, BL], F32, tag="idt", name="idt")
                    nc.scalar.activation(idt[:], ps[:], AF.Identity, scale=0.55)
                    nc.vector.scalar_tensor_tensor(c_d[:, m * BL:(m + 1) * BL],
                                                   ab[:], 0.45, idt[:],
                                                   ALU.mult, ALU.add)

            if phases < 4:
                raise _Stop
            # mem_norm transposes for attention (only need mem_T): spread
            # through the decoder loop to fill PE/ACT gaps
            memN = big.tile([S, BL, 2 * KE, 128], BF16, tag="memN", name="memN")

            # ---------- phase 4: decoder recurrence ----------
            # slab m = gate*4 + j; gates [i, f, o, g]; h/c cols k*4+b
            GW = KD * BL  # 16
            with tc.tile_pool(name="dec_ps", bufs=2, space="PSUM") as dps, \
                 tc.tile_pool(name="dec_g", bufs=2) as dgp, \
                 tc.tile_pool(name="mn_ps", bufs=2, space="PSUM") as mnpool:
                for step in range(T):
                    if step % 8 == 4 and step // 8 < 2 * KE * BL:
                        idx = step // 8
                        b, kd = idx // (2 * KE), idx % (2 * KE)
                        mn_ps = mnpool.tile([S, 128], BF16, tag="mn", name="mn",
                                            padded_shape=[128, 128])
                        nc.tensor.transpose(mn_ps[:], mem_T[:, kd, b, :],
                                            ident[:])
                        nc.scalar.copy(memN[:, b, kd, :], mn_ps[:])
                    hcur = h_d[step % 2]
                    hnxt = h_d[(step + 1) % 2]
                    psf = dps.tile([128, 512], F32, tag="gd", name="gd")
                    ps = psf[:, 0:4 * GW]
                    # x-side: single wide matmul (rhs = whole step's xproj
                    # block); start=True marks the tile's zero-region once
                    nc.tensor.matmul(ps[:], id_q[:], xp_d[:, step, :, :],
                                     start=True, stop=False,
                                     skip_group_check=True)
                    for m in range(MD):
                        sl = ps[:, m * BL:(m + 1) * BL]
                        for k in range(KD):
                            rhs = hcur[:, k * BL:(k + 1) * BL]
                            lt = whh_d_s[:, (m * KD + k) * 128:
                                         (m * KD + k + 1) * 128]
                            nc.tensor.matmul(sl, lt, rhs,
                                             start=False, stop=(k == KD - 1),
                                             skip_group_check=True)
                    g = dgp.tile([128, 4 * GW], F32, tag="ggd", name="ggd")
                    nc.scalar.activation(g[:], ps[:], AF.Sigmoid,
                                         scale=desc_s[:, 0:1])
                    t1 = dgp.tile([128, GW], F32, tag="t1d", name="t1d")
                    nc.vector.tensor_tensor(t1[:], g[:, GW:2 * GW], c_d[:], ALU.mult)
                    u = dgp.tile([128, GW], F32, tag="ud", name="ud")
                    nc.vector.tensor_tensor(u[:], g[:, 0:GW], g[:, 3 * GW:4 * GW],
                                            ALU.mult)
                    t2 = dgp.tile([128, GW], F32, tag="t2d", name="t2d")
                    nc.vector.scalar_tensor_tensor(t2[:], u[:], 2.0, g[:, 0:GW],
                                                   ALU.mult, ALU.subtract)
                    nc.vector.tensor_tensor(c_d[:], t1[:], t2[:], ALU.add)
                    tc_t = dgp.tile([128, GW], FP16, tag="tcd", name="tcd")
                    nc.scalar.activation(tc_t[:], c_d[:], AF.Tanh)
                    nc.vector.scalar_tensor_tensor(hnxt[:], g[:, 2 * GW:3 * GW],
                                                   SH, tc_t[:], ALU.mult, ALU.mult)
                    hb = dgp.tile([128, GW], BF16, tag="hbd", name="hbd")
                    nc.gpsimd.tensor_tensor(hb[:], g[:, 2 * GW:3 * GW], tc_t[:],
                                            ALU.mult)
                    nc.gpsimd.tensor_copy(
                        H_T[:, :, :, step],
                        hb[:].rearrange("p (k b) -> p k b", k=KD))

            if phases < 5:
                raise _Stop
            # ---------- phase 5: attention + FFN ----------
            ctx_blk = big.tile([128, KD, BL * T], BF16, tag="ctx", name="ctx")
            mid_T = big.tile([128, KD, BL * T], BF16, tag="mid_T", name="mid_T")
            with tc.tile_pool(name="at_ps", bufs=2, space="PSUM") as aps, \
                 tc.tile_pool(name="pt_ps", bufs=2, space="PSUM") as pps, \
                 tc.tile_pool(name="cx_ps", bufs=2, space="PSUM") as cps2, \
                 tc.tile_pool(name="md_ps", bufs=2, space="PSUM") as mps, \
                 tc.tile_pool(name="at_sb", bufs=3) as asb:
                for b in range(BL):
                    a_ps = aps.tile([T, S], F32, tag="a", name="a")
                    for kd in range(2 * KE):
                        nc.tensor.matmul(a_ps[:], H_T[:, kd, b, :],
                                         mem_T[:, kd, b, :],
                                         start=(kd == 0), stop=(kd == 2 * KE - 1))
                    mx = asb.tile([T, 1], F32, tag="mx", name="mx")
                    nc.vector.tensor_reduce(mx[:], a_ps[:], AX.X, ALU.max,
                                            negate=True)
                    mx2 = asb.tile([T, 1], F32, tag="mx2", name="mx2")
                    nc.scalar.mul(mx2[:], mx[:], SCALE)
                    ex = asb.tile([T, S], F32, tag="ex", name="ex")
                    den = asb.tile([T, 1], F32, tag="den", name="den")
                    nc.scalar.activation(ex[:], a_ps[:], AF.Exp, bias=mx2[:],
                                         scale=SCALE, accum_out=den[:])
                    rec = asb.tile([T, 1], F32, tag="rec", name="rec")
                    nc.vector.reciprocal(rec[:], den[:])
                    p_sb = asb.tile([T, S], BF16, tag="p", name="p")
                    nc.vector.tensor_scalar_mul(p_sb[:], ex[:], rec[:])
                    pt_ps = pps.tile([S, T], BF16, tag="pt", name="pt",
                                     padded_shape=[128, 128])
                    nc.tensor.transpose(pt_ps[:], p_sb[:], ident[0:T, 0:T])
                    pt_sb = asb.tile([S, T], BF16, tag="pts", name="pts")
                    nc.scalar.copy(pt_sb[:], pt_ps[:])
                    for kd in range(KD):
                        c_ps = cps2.tile([128, T], F32, tag="c", name="c")
                        nc.tensor.matmul(c_ps[:], memN[:, b, kd, :], pt_sb[:],
                                         start=True, stop=True)
                        nc.vector.tensor_copy(
                            ctx_blk[:, kd, b * T:(b + 1) * T], c_ps[:])
                # FFN: mid = lrelu([H; ctx] @ W_f1.T + b1)
                for m in range(KD):
                    ps = mps.tile([128, BL * T], F32, tag="md", name="md")
                    for k in range(KD):
                        lt = wf1_s[:, (m * 8 + k) * 128:(m * 8 + k + 1) * 128]
                        nc.tensor.matmul(ps[:], lt, H_T[:, k], start=(k == 0),
                                         stop=False)
                    for k in range(KD):
                        lt = wf1_s[:, (m * 8 + KD + k) * 128:(m * 8 + KD + k + 1) * 128]
                        nc.tensor.matmul(ps[:], lt, ctx_blk[:, k, :],
                                         start=False, stop=(k == KD - 1))
                    ab = asb.tile([128, BL * T], F32, tag="mab", name="mab")
                    nc.scalar.activation(ab[:], ps[:], AF.Abs,
                                         bias=b1a_s[:, m:m + 1])
                    idt = asb.tile([128, BL * T], F32, tag="mid", name="mid")
                    nc.scalar.activation(idt[:], ps[:], AF.Identity, scale=0.55,
                                         bias=b1h_s[:, m:m + 1])
                    nc.vector.scalar_tensor_tensor(mid_T[:, m, :], ab[:], 0.45,
                                                   idt[:], ALU.mult, ALU.add)

            if phases < 6:
                raise _Stop
            # ---------- phase 6: vocab projection ----------
            wf2_3d = wf2.ap().rearrange("p (k v) -> p k v", k=KD)
            NWC = VQ // WC  # wf2 chunks per v-quarter
            SUB = 500
            NSUB = WC // SUB
            with tc.tile_pool(name="lg_ps", bufs=8, space="PSUM") as lps, \
                 tc.tile_pool(name="wf2_sb", bufs=2) as wfp, \
                 tc.tile_pool(name="lg_sb", bufs=2) as lgp:
                for vq in range(V // VQ):
                    lg = [lgp.tile([128, VQ], BF16, tag=f"lg{b}", name=f"lg{b}")
                          for b in range(BL)]
                    for wc in range(NWC):
                        v0 = vq * VQ + wc * WC
                        wf2c = wfp.tile([128, KD, WC], BF16, tag="wf2c",
                                        name="wf2c")
                        nc.sync.dma_start(wf2c[:], wf2_3d[:, :, v0:v0 + WC])
                        for b in range(BL):
                            pss = []
                            for sub in range(NSUB):
                                pss.append(lps.tile([128, SUB], F32, tag="lg",
                                                    name="lg"))
                            for k in range(KD):
                                lt = mid_T[:, k, b * T:(b + 1) * T]
                                for sub in range(NSUB):
                                    nc.tensor.matmul(
                                        pss[sub][:], lt,
                                        wf2c[:, k, sub * SUB:(sub + 1) * SUB],
                                        start=(k == 0), stop=(k == KD - 1))
                            for sub in range(NSUB):
                                dst = lg[b][:, wc * WC + sub * SUB:
                                            wc * WC + (sub + 1) * SUB]
                                nc.vector.tensor_copy(dst, pss[sub][:])
                    for b in range(BL):
                        nc.scalar.dma_start(
                            out.ap()[b, :, vq * VQ:(vq + 1) * VQ], lg[b][:])

        except _Stop:
            pass
    nc.compile()
    return nc


def prep_inputs(i, use_fp8=True, fp16_h=True):
    """Host-side staging -> list of 8 per-core in_maps."""
    def as_np(x, dt=np.float32):
        return np.ascontiguousarray(np.asarray(x), dtype=dt)

    wdt = fp8 if use_fp8 else bf16
    hdt = fp16 if fp16_h else wdt

    tok = as_np(i["tok_emb"])                  # [V, 128] f32
    inp = as_np(i["inp"], np.int64)            # [B, S]
    x = as_np(i["x"], np.int64)                # [B, T]
    start = as_np(i["start_emb"])[0]           # [128]
    est = as_np(i["enc_style_emb"])            # [2, 512]
    sty = as_np(i["style_emb"])                # [2, 512]
    lab_i = as_np(i["label_i"], np.int64)
    lab = as_np(i["label"], np.int64)

    # ---- weight scale: sw (power of 2), sh fixed ----
    absmax = max(float(np.abs(as_np(i["Whh_d"])).max()),
                 float(np.abs(as_np(i["Whh_f"])).max()),
                 float(np.abs(as_np(i["Whh_b"])).max()), 1e-30)
    sw = 32.0
    while sw * absmax > 224.0 and sw > 1.0 / 65536.0:
        sw /= 2.0
    if not use_fp8:
        sw = 1.0
    descale = np.full((128, 1), 1.0 / (sw * SH), np.float32)
    idscale = np.full((128, 1), sw * SH, np.float32)

    # ---- gate reorder [i, f, o, g] (torch order is i, f, g, o) ----
    # g-gate rows are pre-scaled x2: the kernel computes all four gate
    # nonlinearities with ONE sigmoid op and recovers
    # tanh(g) = 2*sigmoid(2g) - 1 on the DVE (exact identity).
    def reorder_gates(w, H):
        # w [4H, X]: rows [i(H), f(H), g(H), o(H)] -> [i, f, o, 2*g]
        return np.concatenate([w[0:H], w[H:2 * H], w[3 * H:4 * H],
                               2.0 * w[2 * H:3 * H]], axis=0)

    def reorder_bias(v, H):
        return np.concatenate([v[0:H], v[H:2 * H], v[3 * H:4 * H],
                               2.0 * v[2 * H:3 * H]])

    # ---- decoder weights ----
    Wr = reorder_gates(as_np(i["Wih_d"]), D_DEC)          # [2048, 128]
    wih_d = np.ascontiguousarray(
        Wr.reshape(MD, 128, 128).transpose(0, 2, 1)       # (m, p_in, c)
        .transpose(1, 0, 2).reshape(128, MD * 128)).astype(bf16)
    Wr = reorder_gates(as_np(i["Whh_d"]), D_DEC) * sw     # [2048, 512]
    a = Wr.reshape(MD, 128, KD, 128)                      # [m, c, k, p]
    whh_d = np.ascontiguousarray(
        a.transpose(3, 0, 2, 1).reshape(128, MD * KD * 128)).astype(wdt)

    # ---- encoder weights (dirs fused): slab q = gate*4 + dir*2 + j ----
    def enc_tiles(Wf, Wb, ncols, dt, scale=1.0):
        # W* [1024, ncols] (torch gate order); returns [128, QE*(ncols/128)*128]
        Wf = reorder_gates(as_np(Wf), D_ENC) * scale      # [1024, ncols]
        Wb = reorder_gates(as_np(Wb), D_ENC) * scale
        nk = ncols // 128
        tiles = []
        for gate in range(4):
            for d in range(2):
                W = Wf if d == 0 else Wb
                for j in range(KE):
                    rows = W[gate * D_ENC + j * 128: gate * D_ENC + (j + 1) * 128]
                    for k in range(nk):
                        tiles.append(rows[:, k * 128:(k + 1) * 128].T)  # [p, c]
        out_ = np.stack(tiles, axis=1).reshape(128, -1)
        return np.ascontiguousarray(out_).astype(dt)

    wih_e = enc_tiles(i["Wih_f"], i["Wih_b"], 128, bf16)
    whh_e = enc_tiles(i["Whh_f"], i["Whh_b"], D_ENC, wdt, scale=sw)

    # ---- W_tr tiles (m dec-chunk, kd enc-concat chunk; kd-inner) ----
    wtr_w = as_np(i["W_tr"])                               # [512, 512]
    a = wtr_w.reshape(KD, 128, KD, 128)                    # [m, c, kd, p]
    wtr = np.ascontiguousarray(
        a.transpose(3, 0, 2, 1).reshape(128, KD * KD * 128)).astype(fp16)

    # ---- W_f1 tiles (m, k) k-inner; k 0..3 h-part, 4..7 ctx ----
    wf1_w = as_np(i["W_f1"])                               # [512, 1536? 512x(1024+512)?]
    # W_f1 [D_DEC, 2*D_ENC + D_DEC] = [512, 1024]; order [h? ...]
    # reference: ffn = concat([h, ctx]) @ W_f1.T -> cols 0:512 h, 512:1024 ctx
    a = wf1_w.reshape(KD, 128, 8, 128)                     # [m, c, k, p]
    wf1 = np.ascontiguousarray(
        a.transpose(3, 0, 2, 1).reshape(128, 4 * 8 * 128)).astype(bf16)

    # ---- W_f2 [V, 512] -> [128, KD*V] (k-major) ----
    wf2_w = as_np(i["W_f2"])                               # [V, 512]
    a = wf2_w.reshape(V, KD, 128)                          # [v, k, p]
    wf2 = np.ascontiguousarray(a.transpose(2, 1, 0).reshape(128, KD * V)
                               ).astype(bf16)

    # ---- biases ----
    def bias_cols(v, H, nm):
        vr = reorder_bias(v, H)
        return np.ascontiguousarray(vr.reshape(nm, 128).T)  # [128, nm]

    bs_d = bias_cols(as_np(i["bih_d"]) + as_np(i["bhh_d"]), D_DEC, MD)
    # encoder fused: slab q = gate*4 + dir*2 + j
    be_f = reorder_bias(as_np(i["bih_f"]) + as_np(i["bhh_f"]), D_ENC).reshape(4, KE, 128)
    be_b = reorder_bias(as_np(i["bih_b"]) + as_np(i["bhh_b"]), D_ENC).reshape(4, KE, 128)
    bs_e = np.zeros((128, QE), np.float32)
    for gate in range(4):
        for d in range(2):
            for j in range(KE):
                q = gate * 4 + d * 2 + j
                bs_e[:, q] = (be_f if d == 0 else be_b)[gate, j]
    b1 = as_np(i["b_f1"])
    b1a = np.ascontiguousarray(b1.reshape(KD, 128).T)
    b1h = np.ascontiguousarray((0.55 * b1).reshape(KD, 128).T)

    in_maps = []
    for c in range(N_CORES):
        bs = slice(c * BL, (c + 1) * BL)
        # encoder token embeddings: [128, s*BL+b]
        e = tok[inp[bs]]                                   # [BL, S, 128]
        embT_e = np.ascontiguousarray(
            e.transpose(2, 1, 0).reshape(128, S * BL)).astype(bf16)
        # decoder teacher inputs: start, x[:, :-1]
        d = np.empty((BL, T, D_EMB), np.float32)
        d[:, 0, :] = start
        d[:, 1:, :] = tok[x[bs, :T - 1]]
        embT_d = np.ascontiguousarray(
            d.transpose(2, 1, 0).reshape(128, T * BL)).astype(bf16)
        # encoder init h: [128, dir*8 + k*4 + b] scaled
        h0 = est[lab_i[bs]].reshape(BL, 2, KE, 128)        # [b, dir, k, p]
        h0e = np.ascontiguousarray(
            (h0.transpose(3, 1, 2, 0) * SH).reshape(128, 2 * KE * BL)).astype(hdt)
        # decoder init h: [128, k*4+b] scaled
        ht = sty[lab[bs]].reshape(BL, KD, 128)
        ht0 = np.ascontiguousarray(
            (ht.transpose(2, 1, 0) * SH).reshape(128, KD * BL)).astype(hdt)
        in_maps.append(dict(
            embT_e=embT_e, embT_d=embT_d, h0e_q=h0e, ht0_q=ht0,
            wih_e=wih_e, wih_d=wih_d, whh_e=whh_e, whh_d=whh_d,
            wtr=wtr, wf1=wf1, wf2=wf2,
            bs_e=bs_e, bs_d=bs_d, b1a=b1a, b1h=b1h,
            descale=descale, idscale=idscale))
    return in_maps


_NC_CACHE = {}


def kernel(**inputs):
    bias_free = all(
        not np.any(np.asarray(inputs[k]))
        for k in ("bih_f", "bhh_f", "bih_b", "bhh_b", "bih_d", "bhh_d"))
    key = f"full{bias_free}"
    if key not in _NC_CACHE:
        _NC_CACHE[key] = build(bias_free=bias_free)
    nc = _NC_CACHE[key]
    in_maps = prep_inputs(inputs)
    res = run_bass_kernel_spmd(nc, in_maps, core_ids=list(range(N_CORES)))
    return np.concatenate(
        [r["out"].astype(np.float32) for r in res.results], axis=0)


# revision 43
# speedup vs baseline: 1.0228x; 1.0114x over previous
"""DenoiseLSTM Trainium2 kernel (8 NeuronCores, SPMD, batch-parallel).

Strategy: fully data-parallel over batch — each core runs the ENTIRE network
(bi-LSTM encoder, LSTM decoder with attention, FFN, vocab projection) for its
4 of the 32 batches. No collectives; the host concatenates batch shards.

Key optimizations over the vocab-sharded baseline:
- All elementwise/softmax/FFN work shrinks 8x per core (B_local=4), so the
  LSTM steps' serial DVE/ACT chain is ~2.6x shorter per step.
- Recurrence weights quantized to fp8e4 (x sw=32); hidden state kept in
  fp16 (x sh=128) — mixed fp8-lhsT x fp16-rhs matmuls are HW-verified.
  fp8 weights halve the per-step LDWEIGHTS stream on hardware (FWL reads
  4 fp8/cycle), which is the decoder's throughput floor and is NOT modeled
  by the CoreSim cost model. Descale (1/(sw*sh)) is folded into the gate
  activations' `scale` operand.
- The x-projection contribution enters the gate PSUM through one extra wide
  matmul per step (lhsT = (sw*sh)*Identity, rhs = the step's whole xproj
  block), removing both the DVE add and 15 weight reloads from the step.
  PSUM zero-region semantics: only the first matmul per gate tile carries
  start=True, and gate PSUM tiles are padded to a full 2KB bank.
- ALL four gate nonlinearities run as ONE sigmoid op: g-gate weights are
  pre-scaled x2 on the host and tanh(g) = 2*sigmoid(2g) - 1 is recovered on
  the DVE (exact identity; intermediates kept f32 to avoid cancellation).
  Off-critical stores (h->memory/H) run on the otherwise idle GPSIMD
  (SBUF-only: GPSIMD cannot access PSUM).
- Token/style embedding gathers are host-side indexing; no device gathers.
- Vocab projection streams the full W_f2 [32000,512] once (bf16) while
  accumulating bf16 logits in SBUF (PE ~99% busy at the N-column roofline);
  wf2-in DMAs ride the sync queue, logits-out the scalar queue; output is
  bf16 on device and upcast to f32 on host.
"""
import sys

sys.path.insert(0, "/opt/trn_rl_repo")

from contextlib import ExitStack

import numpy as np
import ml_dtypes

import concourse.bass as bass
import concourse.bacc as bacc
import concourse.mybir as mybir
import concourse.tile as tile
from concourse.bass_utils import run_bass_kernel_spmd
from concourse.masks import make_identity

bf16 = ml_dtypes.bfloat16
fp16 = np.float16
fp8 = ml_dtypes.float8_e4m3
F32 = mybir.dt.float32
BF16 = mybir.dt.bfloat16
FP16 = mybir.dt.float16
F8 = mybir.dt.float8e4
AF = mybir.ActivationFunctionType
ALU = mybir.AluOpType
AX = mybir.AxisListType

B = 32
BL = 4                 # batches per core
D_EMB = 128
D_ENC = 256
D_DEC = 512
N_CORES = 8
S = 128
T = 128
V = 32000

KD = 4                 # decoder hidden chunks (512/128)
MD = 16                # decoder gate chunks (2048/128)
KE = 2                 # encoder hidden chunks per direction
QE = 16                # encoder fused gate slabs: gate(4) x dir(2) x j(2)
SH = 128.0             # hidden-state fp8 scale (compile-time)

VQ = 4000              # vocab output block
WC = 1000              # wf2 streaming chunk (v columns)


class _Stop(Exception):
    pass


def build(phases=7, use_fp8=True, fp16_h=True, bias_free=True):
    nc = bacc.Bacc("TRN2", target_bir_lowering=False, debug=False)
    WDT = F8 if use_fp8 else BF16
    HDT = FP16 if fp16_h else WDT
    SCALE = 1.0 / float(np.sqrt(np.float32(2 * D_ENC)))

    # ---- external inputs (host-prepped, per core) ----
    embT_e = nc.dram_tensor("embT_e", [128, S * BL], BF16, kind="ExternalInput")
    embT_d = nc.dram_tensor("embT_d", [128, T * BL], BF16, kind="ExternalInput")
    h0e_q = nc.dram_tensor("h0e_q", [128, 2 * KE * BL], HDT, kind="ExternalInput")
    ht0_q = nc.dram_tensor("ht0_q", [128, KD * BL], HDT, kind="ExternalInput")
    wih_e = nc.dram_tensor("wih_e", [128, QE * 128], BF16, kind="ExternalInput")
    wih_d = nc.dram_tensor("wih_d", [128, MD * 128], BF16, kind="ExternalInput")
    whh_e = nc.dram_tensor("whh_e", [128, QE * KE * 128], WDT, kind="ExternalInput")
    whh_d = nc.dram_tensor("whh_d", [128, MD * KD * 128], WDT, kind="ExternalInput")
    wtr = nc.dram_tensor("wtr", [128, KD * KD * 128], FP16, kind="ExternalInput")
    wf1 = nc.dram_tensor("wf1", [128, 4 * 8 * 128], BF16, kind="ExternalInput")
    wf2 = nc.dram_tensor("wf2", [128, KD * V], BF16, kind="ExternalInput")
    bs_e = nc.dram_tensor("bs_e", [128, QE], F32, kind="ExternalInput")
    bs_d = nc.dram_tensor("bs_d", [128, MD], F32, kind="ExternalInput")
    b1a = nc.dram_tensor("b1a", [128, KD], F32, kind="ExternalInput")
    b1h = nc.dram_tensor("b1h", [128, KD], F32, kind="ExternalInput")
    descale = nc.dram_tensor("descale", [128, 1], F32, kind="ExternalInput")
    idscale = nc.dram_tensor("idscale", [128, 1], F32, kind="ExternalInput")

    out = nc.dram_tensor("out", [BL, T, V], BF16, kind="ExternalOutput")

    with tile.TileContext(nc) as tc, ExitStack() as ctx:
        wpool = ctx.enter_context(tc.tile_pool(name="weights", bufs=1))
        spool = ctx.enter_context(tc.tile_pool(name="state", bufs=1))
        big = ctx.enter_context(tc.tile_pool(name="big", bufs=1))

        def load(dram, shape, dtype, tag, eng=None):
            t = wpool.tile(shape, dtype, tag=tag, name=tag)
            (eng or nc.sync).dma_start(t[:], dram[:, :])
            return t

        # all loads on the sync queue, in need-order: encoder-critical
        # first (each dma_start costs ~565ns on the shared DGE sequencer,
        # and big transfers queue behind earlier ones), then decoder-side
        desc_s = load(descale, [128, 1], F32, "descale")
        idsc_s = load(idscale, [128, 1], F32, "idscale")
        embT_e_s = load(embT_e, [128, S * BL], BF16, "embT_e")
        wih_e_s = load(wih_e, [128, QE * 128], BF16, "wih_e")
        whh_e_s = load(whh_e, [128, QE * KE * 128], WDT, "whh_e")
        bs_e_s = load(bs_e, [128, QE], F32, "bs_e")

        # ---- state tiles (initial h DMAs early: encoder step 0 needs them)
        h_e = [spool.tile([128, 2 * KE * BL], HDT, tag=f"h_e{j}", name=f"h_e{j}")
               for j in range(2)]
        c_e = spool.tile([128, 2 * KE * BL], FP16, tag="c_e", name="c_e")
        h_d = [spool.tile([128, KD * BL], HDT, tag=f"h_d{j}", name=f"h_d{j}")
               for j in range(2)]
        c_d = spool.tile([128, KD * BL], FP16, tag="c_d", name="c_d")
        nc.sync.dma_start(h_e[0][:], h0e_q[:, :])
        nc.sync.dma_start(h_d[0][:], ht0_q[:, :])
        nc.vector.memset(c_e[:], 0.0)

        embT_d_s = load(embT_d, [128, T * BL], BF16, "embT_d")
        wih_d_s = load(wih_d, [128, MD * 128], BF16, "wih_d")
        whh_d_s = load(whh_d, [128, MD * KD * 128], WDT, "whh_d")
        wtr_s = load(wtr, [128, KD * KD * 128], FP16, "wtr")
        wf1_s = load(wf1, [128, 4 * 8 * 128], BF16, "wf1")
        bs_d_s = load(bs_d, [128, MD], F32, "bs_d")
        b1a_s = load(b1a, [128, KD], F32, "b1a")
        b1h_s = load(b1h, [128, KD], F32, "b1h")

        ident = wpool.tile([128, 128], BF16, tag="ident", name="ident")
        make_identity(nc, ident)
        # identity scaled by sw*sh: x-projection enters PSUM pre-scaled so the
        # gate activation descale (1/(sw*sh)) recovers the true pre-activation
        id_q = wpool.tile([128, 128], BF16, tag="id_q", name="id_q")
        nc.scalar.activation(id_q[:], ident[:], AF.Copy, scale=idsc_s[:, 0:1])

        # xproj: [128, s, q, b] so a step's rhs block is [128, q, b]
        xp_e = big.tile([128, S, QE, BL], BF16, tag="xp_e", name="xp_e")
        xp_d = big.tile([128, T, MD, BL], BF16, tag="xp_d", name="xp_d")
        # memory (encoder h) and decoder h, [128, kd, b, s/t]
        mem_T = big.tile([128, 2 * KE, BL, S], BF16, tag="mem_T", name="mem_T")
        H_T = big.tile([128, KD, BL, T], BF16, tag="H_T", name="H_T")

        # ---------- phase 1: encoder x-projection ----------
        # (decoder x-projection is emitted after the encoder loop so the
        # scheduler can run it inside encoder PE/DVE gaps)
        with tc.tile_pool(name="xp_ps", bufs=4, space="PSUM") as xps:
            def proj(w_s, src, n_q, bias_s, dst):
                for q in range(n_q):
                    ps = xps.tile([128, 512], F32, tag="xp", name="xp")
                    nc.tensor.matmul(ps[:], w_s[:, q * 128:(q + 1) * 128],
                                     src[:], start=True, stop=True)
                    ps3 = ps[:].rearrange("p (s b) -> p s b", b=BL)
                    if bias_free and q % 2 == 0:
                        nc.scalar.copy(dst[:, :, q, :], ps3)
                    elif bias_free:
                        nc.vector.tensor_copy(dst[:, :, q, :], ps3)
                    else:
                        nc.vector.tensor_scalar_add(dst[:, :, q, :], ps3,
                                                    bias_s[:, q:q + 1])

            proj(wih_e_s, embT_e_s, QE, bs_e_s, xp_e)

        try:
            if phases < 2:
                raise _Stop
            # ---------- phase 2: encoder recurrence (dirs fused) ----------
            # slab q = gate*4 + dir*2 + j;  gates ordered [i, f, o, g]
            # gate tile cols: i [0:16], f [16:32], o [32:48], g [48:64]
            # h/c cols: dir*8 + k*4 + b
            GW = 16
            with tc.tile_pool(name="enc_ps", bufs=2, space="PSUM") as eps, \
                 tc.tile_pool(name="enc_g", bufs=2) as egp:
                for step in range(S):
                    hcur = h_e[step % 2]
                    hnxt = h_e[(step + 1) % 2]
                    # full 2KB bank per gate tile: a start=True marks the
                    # whole PSUM zero-region, so tiles must not share banks
                    psf = eps.tile([128, 512], F32, tag="g", name="g")
                    ps = psf[:, 0:4 * GW]
                    # x-side: one matmul per (gate, dir) — contiguous 8-col
                    # out slabs; the first start=True marks the zero-region
                    for gv in range(4):
                        for dirv in range(2):
                            s_idx = step if dirv == 0 else S - 1 - step
                            q0 = gv * 4 + dirv * 2
                            nc.tensor.matmul(
                                ps[:, q0 * BL:(q0 + 2) * BL], id_q[:],
                                xp_e[:, s_idx, q0:q0 + 2, :],
                                start=(gv == 0 and dirv == 0), stop=False,
                                skip_group_check=True)
                    for q in range(QE):
                        d = (q >> 1) & 1
                        sl = ps[:, q * BL:(q + 1) * BL]
                        for k in range(KE):
                            rhs = hcur[:, (d * KE + k) * BL:(d * KE + k + 1) * BL]
                            lt = whh_e_s[:, (q * KE + k) * 128:(q * KE + k + 1) * 128]
                            nc.tensor.matmul(sl, lt, rhs,
                                             start=False, stop=(k == KE - 1),
                                             skip_group_check=True)
                    # gate tile f32: the i*tanh(g) = 2*(i*sig2g) - i identity
                    # cancels near 0.5, so intermediates must be f32
                    g = egp.tile([128, 4 * GW], F32, tag="gg", name="gg")
                    nc.scalar.activation(g[:], ps[:], AF.Sigmoid,
                                         scale=desc_s[:, 0:1])
                    t1 = egp.tile([128, GW], F32, tag="t1", name="t1")
                    nc.vector.tensor_tensor(t1[:], g[:, GW:2 * GW], c_e[:], ALU.mult)
                    u = egp.tile([128, GW], F32, tag="u", name="u")
                    nc.vector.tensor_tensor(u[:], g[:, 0:GW], g[:, 3 * GW:4 * GW],
                                            ALU.mult)
                    t2 = egp.tile([128, GW], F32, tag="t2", name="t2")
                    nc.vector.scalar_tensor_tensor(t2[:], u[:], 2.0, g[:, 0:GW],
                                                   ALU.mult, ALU.subtract)
                    nc.vector.tensor_tensor(c_e[:], t1[:], t2[:], ALU.add)
                    tc_t = egp.tile([128, GW], FP16, tag="tc", name="tc")
                    nc.scalar.activation(tc_t[:], c_e[:], AF.Tanh)
                    nc.vector.scalar_tensor_tensor(hnxt[:], g[:, 2 * GW:3 * GW],
                                                   SH, tc_t[:], ALU.mult, ALU.mult)
                    hb = egp.tile([128, GW], BF16, tag="hb", name="hb")
                    nc.gpsimd.tensor_tensor(hb[:], g[:, 2 * GW:3 * GW], tc_t[:],
                                            ALU.mult)
                    nc.gpsimd.tensor_copy(
                        mem_T[:, 0:KE, :, step],
                        hb[:, 0:KE * BL].rearrange("p (k b) -> p k b", k=KE))
                    nc.gpsimd.tensor_copy(
                        mem_T[:, KE:2 * KE, :, S - 1 - step],
                        hb[:, KE * BL:2 * KE * BL].rearrange("p (k b) -> p k b", k=KE))

            # decoder x-projection (independent of the encoder: fills gaps)
            with tc.tile_pool(name="xpd_ps", bufs=4, space="PSUM") as xps2:
                for q in range(MD):
                    ps = xps2.tile([128, 512], F32, tag="xpd", name="xpd")
                    nc.tensor.matmul(ps[:], wih_d_s[:, q * 128:(q + 1) * 128],
                                     embT_d_s[:], start=True, stop=True)
                    ps3 = ps[:].rearrange("p (s b) -> p s b", b=BL)
                    if bias_free and q % 2 == 0:
                        nc.scalar.copy(xp_d[:, :, q, :], ps3)
                    elif bias_free:
                        nc.vector.tensor_copy(xp_d[:, :, q, :], ps3)
                    else:
                        nc.vector.tensor_scalar_add(xp_d[:, :, q, :], ps3,
                                                    bs_d_s[:, q:q + 1])

            if phases < 3:
                raise _Stop
            # ---------- phase 3: c_t init (W_tr @ [cf;cb], lrelu) ----------
            with tc.tile_pool(name="ct_ps", bufs=2, space="PSUM") as cps, \
                 tc.tile_pool(name="ct_sb", bufs=2) as csb:
                for m in range(KD):
                    ps = cps.tile([128, BL], F32, tag="ct", name="ct")
                    for k in range(KD):
                        lt = wtr_s[:, (m * KD + k) * 128:(m * KD + k + 1) * 128]
                        nc.tensor.matmul(ps[:], lt, c_e[:, k * BL:(k + 1) * BL],
                                         start=(k == 0), stop=(k == KD - 1))
                    ab = csb.tile([128, BL], F32, tag="ab", name="ab")
                    nc.scalar.activation(ab[:], ps[:], AF.Abs)
                    idt = csb.tile([128, BL], F32, tag="idt", name="idt")
                    nc.scalar.activation(idt[:], ps[:], AF.Identity, scale=0.55)
                    nc.vector.scalar_tensor_tensor(c_d[:, m * BL:(m + 1) * BL],
                                                   ab[:], 0.45, idt[:],
                                                   ALU.mult, ALU.add)

            if phases < 4:
                raise _Stop
            # mem_norm transposes for attention (only need mem_T): spread
            # through the decoder loop to fill PE/ACT gaps
            memN = big.tile([S, BL, 2 * KE, 128], BF16, tag="memN", name="memN")

            # ---------- phase 4: decoder recurrence ----------
            # slab m = gate*4 + j; gates [i, f, o, g]; h/c cols k*4+b
            GW = KD * BL  # 16
            with tc.tile_pool(name="dec_ps", bufs=2, space="PSUM") as dps, \
                 tc.tile_pool(name="dec_g", bufs=2) as dgp, \
                 tc.tile_pool(name="mn_ps", bufs=2, space="PSUM") as mnpool:
                for step in range(T):
                    if step % 8 == 4 and step // 8 < 2 * KE * BL:
                        idx = step // 8
                        b, kd = idx // (2 * KE), idx % (2 * KE)
                        mn_ps = mnpool.tile([S, 128], BF16, tag="mn", name="mn",
                                            padded_shape=[128, 128])
                        nc.tensor.transpose(mn_ps[:], mem_T[:, kd, b, :],
                                            ident[:])
                        nc.scalar.copy(memN[:, b, kd, :], mn_ps[:])
                    hcur = h_d[step % 2]
                    hnxt = h_d[(step + 1) % 2]
                    psf = dps.tile([128, 512], F32, tag="gd", name="gd")
                    ps = psf[:, 0:4 * GW]
                    # x-side: single wide matmul (rhs = whole step's xproj
                    # block); start=True marks the tile's zero-region once
                    nc.tensor.matmul(ps[:], id_q[:], xp_d[:, step, :, :],
                                     start=True, stop=False,
                                     skip_group_check=True)
                    for m in range(MD):
                        sl = ps[:, m * BL:(m + 1) * BL]
                        for k in range(KD):
                            rhs = hcur[:, k * BL:(k + 1) * BL]
                            lt = whh_d_s[:, (m * KD + k) * 128:
                                         (m * KD + k + 1) * 128]
                            nc.tensor.matmul(sl, lt, rhs,
                                             start=False, stop=(k == KD - 1),
                                             skip_group_check=True)
                    g = dgp.tile([128, 4 * GW], F32, tag="ggd", name="ggd")
                    nc.scalar.activation(g[:], ps[:], AF.Sigmoid,
                                         scale=desc_s[:, 0:1])
                    t1 = dgp.tile([128, GW], F32, tag="t1d", name="t1d")
                    nc.vector.tensor_tensor(t1[:], g[:, GW:2 * GW], c_d[:], ALU.mult)
                    u = dgp.tile([128, GW], F32, tag="ud", name="ud")
                    nc.vector.tensor_tensor(u[:], g[:, 0:GW], g[:, 3 * GW:4 * GW],
                                            ALU.mult)
                    t2 = dgp.tile([128, GW], F32, tag="t2d", name="t2d")
                    nc.vector.scalar_tensor_tensor(t2[:], u[:], 2.0, g[:, 0:GW],
                                                   ALU.mult, ALU.subtract)
                    nc.vector.tensor_tensor(c_d[:], t1[:], t2[:], ALU.add)
                    tc_t = dgp.tile([128, GW], FP16, tag="tcd", name="tcd")
                    nc.scalar.activation(tc_t[:], c_d[:], AF.Tanh)
                    nc.vector.scalar_tensor_tensor(hnxt[:], g[:, 2 * GW:3 * GW],
                                                   SH, tc_t[:], ALU.mult, ALU.mult)
                    hb = dgp.tile([128, GW], BF16, tag="hbd", name="hbd")
                    nc.gpsimd.tensor_tensor(hb[:], g[:, 2 * GW:3 * GW], tc_t[:],
                                            ALU.mult)
                    nc.gpsimd.tensor_copy(
                        H_T[:, :, :, step],
                        hb[:].rearrange("p (k b) -> p k b", k=KD))

            if phases < 5:
                raise _Stop
            # ---------- phase 5: attention + FFN ----------
            ctx_blk = big.tile([128, KD, BL * T], BF16, tag="ctx", name="ctx")
            mid_T = big.tile([128, KD, BL * T], BF16, tag="mid_T", name="mid_T")
            with tc.tile_pool(name="at_ps", bufs=2, space="PSUM") as aps, \
                 tc.tile_pool(name="pt_ps", bufs=2, space="PSUM") as pps, \
                 tc.tile_pool(name="cx_ps", bufs=2, space="PSUM") as cps2, \
                 tc.tile_pool(name="md_ps", bufs=2, space="PSUM") as mps, \
                 tc.tile_pool(name="at_sb", bufs=3) as asb:
                for b in range(BL):
                    a_ps = aps.tile([T, S], F32, tag="a", name="a")
                    for kd in range(2 * KE):
                        nc.tensor.matmul(a_ps[:], H_T[:, kd, b, :],
                                         mem_T[:, kd, b, :],
                                         start=(kd == 0), stop=(kd == 2 * KE - 1))
                    mx = asb.tile([T, 1], F32, tag="mx", name="mx")
                    nc.vector.tensor_reduce(mx[:], a_ps[:], AX.X, ALU.max,
                                            negate=True)
                    mx2 = asb.tile([T, 1], F32, tag="mx2", name="mx2")
                    nc.scalar.mul(mx2[:], mx[:], SCALE)
                    ex = asb.tile([T, S], F32, tag="ex", name="ex")
                    den = asb.tile([T, 1], F32, tag="den", name="den")
                    nc.scalar.activation(ex[:], a_ps[:], AF.Exp, bias=mx2[:],
                                         scale=SCALE, accum_out=den[:])
                    rec = asb.tile([T, 1], F32, tag="rec", name="rec")
                    nc.vector.reciprocal(rec[:], den[:])
                    p_sb = asb.tile([T, S], BF16, tag="p", name="p")
                    nc.vector.tensor_scalar_mul(p_sb[:], ex[:], rec[:])
                    pt_ps = pps.tile([S, T], BF16, tag="pt", name="pt",
                                     padded_shape=[128, 128])
                    nc.tensor.transpose(pt_ps[:], p_sb[:], ident[0:T, 0:T])
                    pt_sb = asb.tile([S, T], BF16, tag="pts", name="pts")
                    nc.scalar.copy(pt_sb[:], pt_ps[:])
                    for kd in range(KD):
                        c_ps = cps2.tile([128, T], F32, tag="c", name="c")
                        nc.tensor.matmul(c_ps[:], memN[:, b, kd, :], pt_sb[:],
                                         start=True, stop=True)
                        nc.vector.tensor_copy(
                            ctx_blk[:, kd, b * T:(b + 1) * T], c_ps[:])
                # FFN: mid = lrelu([H; ctx] @ W_f1.T + b1)
                for m in range(KD):
                    ps = mps.tile([128, BL * T], F32, tag="md", name="md")
                    for k in range(KD):
                        lt = wf1_s[:, (m * 8 + k) * 128:(m * 8 + k + 1) * 128]
                        nc.tensor.matmul(ps[:], lt, H_T[:, k], start=(k == 0),
                                         stop=False)
                    for k in range(KD):
                        lt = wf1_s[:, (m * 8 + KD + k) * 128:(m * 8 + KD + k + 1) * 128]
                        nc.tensor.matmul(ps[:], lt, ctx_blk[:, k, :],
                                         start=False, stop=(k == KD - 1))
                    ab = asb.tile([128, BL * T], F32, tag="mab", name="mab")
                    nc.scalar.activation(ab[:], ps[:], AF.Abs,
                                         bias=b1a_s[:, m:m + 1])
                    idt = asb.tile([128, BL * T], F32, tag="mid", name="mid")
                    nc.scalar.activation(idt[:], ps[:], AF.Identity, scale=0.55,
                                         bias=b1h_s[:, m:m + 1])
                    nc.vector.scalar_tensor_tensor(mid_T[:, m, :], ab[:], 0.45,
                                                   idt[:], ALU.mult, ALU.add)

            if phases < 6:
                raise _Stop
            # ---------- phase 6: vocab projection ----------
            wf2_3d = wf2.ap().rearrange("p (k v) -> p k v", k=KD)
            NWC = VQ // WC  # wf2 chunks per v-quarter
            SUB = 500
            NSUB = WC // SUB
            with tc.tile_pool(name="lg_ps", bufs=8, space="PSUM") as lps, \
                 tc.tile_pool(name="wf2_sb", bufs=2) as wfp, \
                 tc.tile_pool(name="lg_sb", bufs=2) as lgp:
                for vq in range(V // VQ):
                    lg = [lgp.tile([128, VQ], BF16, tag=f"lg{b}", name=f"lg{b}")
                          for b in range(BL)]
                    for wc in range(NWC):
                        v0 = vq * VQ + wc * WC
                        wf2c = wfp.tile([128, KD, WC], BF16, tag="wf2c",
                                        name="wf2c")
                        nc.sync.dma_start(wf2c[:], wf2_3d[:, :, v0:v0 + WC])
                        for b in range(BL):
                            pss = []
                            for sub in range(NSUB):
                                pss.append(lps.tile([128, SUB], F32, tag="lg",
                                                    name="lg"))
                            for k in range(KD):
                                lt = mid_T[:, k, b * T:(b + 1) * T]
                                for sub in range(NSUB):
                                    nc.tensor.matmul(
                                        pss[sub][:], lt,
                                        wf2c[:, k, sub * SUB:(sub + 1) * SUB],
                                        start=(k == 0), stop=(k == KD - 1))
                            for sub in range(NSUB):
                                dst = lg[b][:, wc * WC + sub * SUB:
                                            wc * WC + (sub + 1) * SUB]
                                nc.vector.tensor_copy(dst, pss[sub][:])
                    for b in range(BL):
                        nc.scalar.dma_start(
                            out.ap()[b, :, vq * VQ:(vq + 1) * VQ], lg[b][:])

        except _Stop:
            pass
    nc.compile()
    return nc


def prep_inputs(i, use_fp8=True, fp16_h=True):
    """Host-side staging -> list of 8 per-core in_maps."""
    def as_np(x, dt=np.float32):
        return np.ascontiguousarray(np.asarray(x), dtype=dt)

    wdt = fp8 if use_fp8 else bf16
    hdt = fp16 if fp16_h else wdt

    tok = as_np(i["tok_emb"])                  # [V, 128] f32
    inp = as_np(i["inp"], np.int64)            # [B, S]
    x = as_np(i["x"], np.int64)                # [B, T]
    start = as_np(i["start_emb"])[0]           # [128]
    est = as_np(i["enc_style_emb"])            # [2, 512]
    sty = as_np(i["style_emb"])                # [2, 512]
    lab_i = as_np(i["label_i"], np.int64)
    lab = as_np(i["label"], np.int64)

    # ---- weight scale: sw (power of 2), sh fixed ----
    absmax = max(float(np.abs(as_np(i["Whh_d"])).max()),
                 float(np.abs(as_np(i["Whh_f"])).max()),
                 float(np.abs(as_np(i["Whh_b"])).max()), 1e-30)
    sw = 32.0
    while sw * absmax > 224.0 and sw > 1.0 / 65536.0:
        sw /= 2.0
    if not use_fp8:
        sw = 1.0
    descale = np.full((128, 1), 1.0 / (sw * SH), np.float32)
    idscale = np.full((128, 1), sw * SH, np.float32)

    # ---- gate reorder [i, f, o, g] (torch order is i, f, g, o) ----
    # g-gate rows are pre-scaled x2: the kernel computes all four gate
    # nonlinearities with ONE sigmoid op and recovers
    # tanh(g) = 2*sigmoid(2g) - 1 on the DVE (exact identity).
    def reorder_gates(w, H):
        # w [4H, X]: rows [i(H), f(H), g(H), o(H)] -> [i, f, o, 2*g]
        return np.concatenate([w[0:H], w[H:2 * H], w[3 * H:4 * H],
                               2.0 * w[2 * H:3 * H]], axis=0)

    def reorder_bias(v, H):
        return np.concatenate([v[0:H], v[H:2 * H], v[3 * H:4 * H],
                               2.0 * v[2 * H:3 * H]])

    # ---- decoder weights ----
    Wr = reorder_gates(as_np(i["Wih_d"]), D_DEC)          # [2048, 128]
    wih_d = np.ascontiguousarray(
        Wr.reshape(MD, 128, 128).transpose(0, 2, 1)       # (m, p_in, c)
        .transpose(1, 0, 2).reshape(128, MD * 128)).astype(bf16)
    Wr = reorder_gates(as_np(i["Whh_d"]), D_DEC) * sw     # [2048, 512]
    a = Wr.reshape(MD, 128, KD, 128)                      # [m, c, k, p]
    whh_d = np.ascontiguousarray(
        a.transpose(3, 0, 2, 1).reshape(128, MD * KD * 128)).astype(wdt)

    # ---- encoder weights (dirs fused): slab q = gate*4 + dir*2 + j ----
    def enc_tiles(Wf, Wb, ncols, dt, scale=1.0):
        # W* [1024, ncols] (torch gate order); returns [128, QE*(ncols/128)*128]
        Wf = reorder_gates(as_np(Wf), D_ENC) * scale      # [1024, ncols]
        Wb = reorder_gates(as_np(Wb), D_ENC) * scale
        nk = ncols // 128
        tiles = []
        for gate in range(4):
            for d in range(2):
                W = Wf if d == 0 else Wb
                for j in range(KE):
                    rows = W[gate * D_ENC + j * 128: gate * D_ENC + (j + 1) * 128]
                    for k in range(nk):
                        tiles.append(rows[:, k * 128:(k + 1) * 128].T)  # [p, c]
        out_ = np.stack(tiles, axis=1).reshape(128, -1)
        return np.ascontiguousarray(out_).astype(dt)

    wih_e = enc_tiles(i["Wih_f"], i["Wih_b"], 128, bf16)
    whh_e = enc_tiles(i["Whh_f"], i["Whh_b"], D_ENC, wdt, scale=sw)

    # ---- W_tr tiles (m dec-chunk, kd enc-concat chunk; kd-inner) ----
    wtr_w = as_np(i["W_tr"])                               # [512, 512]
    a = wtr_w.reshape(KD, 128, KD, 128)                    # [m, c, kd, p]
    wtr = np.ascontiguousarray(
        a.transpose(3, 0, 2, 1).reshape(128, KD * KD * 128)).astype(fp16)

    # ---- W_f1 tiles (m, k) k-inner; k 0..3 h-part, 4..7 ctx ----
    wf1_w = as_np(i["W_f1"])                               # [512, 1536? 512x(1024+512)?]
    # W_f1 [D_DEC, 2*D_ENC + D_DEC] = [512, 1024]; order [h? ...]
    # reference: ffn = concat([h, ctx]) @ W_f1.T -> cols 0:512 h, 512:1024 ctx
    a = wf1_w.reshape(KD, 128, 8, 128)                     # [m, c, k, p]
    wf1 = np.ascontiguousarray(
        a.transpose(3, 0, 2, 1).reshape(128, 4 * 8 * 128)).astype(bf16)

    # ---- W_f2 [V, 512] -> [128, KD*V] (k-major) ----
    wf2_w = as_np(i["W_f2"])                               # [V, 512]
    a = wf2_w.reshape(V, KD, 128)                          # [v, k, p]
    wf2 = np.ascontiguousarray(a.transpose(2, 1, 0).reshape(128, KD * V)
                               ).astype(bf16)

    # ---- biases ----
    def bias_cols(v, H, nm):
        vr = reorder_bias(v, H)
        return np.ascontiguousarray(vr.reshape(nm, 128).T)  # [128, nm]

    bs_d = bias_cols(as_np(i["bih_d"]) + as_np(i["bhh_d"]), D_DEC, MD)
    # encoder fused: slab q = gate*4 + dir*2 + j
    be_f = reorder_bias(as_np(i["bih_f"]) + as_np(i["bhh_f"]), D_ENC).reshape(4, KE, 128)
    be_b = reorder_bias(as_np(i["bih_b"]) + as_np(i["bhh_b"]), D_ENC).reshape(4, KE, 128)
    bs_e = np.zeros((128, QE), np.float32)
    for gate in range(4):
        for d in range(2):
            for j in range(KE):
                q = gate * 4 + d * 2 + j
                bs_e[:, q] = (be_f if d == 0 else be_b)[gate, j]
    b1 = as_np(i["b_f1"])
    b1a = np.ascontiguousarray(b1.reshape(KD, 128).T)
    b1h = np.ascontiguousarray((0.55 * b1).reshape(KD, 128).T)

    in_maps = []
    for c in range(N_CORES):
        bs = slice(c * BL, (c + 1) * BL)
        # encoder token embeddings: [128, s*BL+b]
        e = tok[inp[bs]]                                   # [BL, S, 128]
        embT_e = np.ascontiguousarray(
            e.transpose(2, 1, 0).reshape(128, S * BL)).astype(bf16)
        # decoder teacher inputs: start, x[:, :-1]
        d = np.empty((BL, T, D_EMB), np.float32)
        d[:, 0, :] = start
        d[:, 1:, :] = tok[x[bs, :T - 1]]
        embT_d = np.ascontiguousarray(
            d.transpose(2, 1, 0).reshape(128, T * BL)).astype(bf16)
        # encoder init h: [128, dir*8 + k*4 + b] scaled
        h0 = est[lab_i[bs]].reshape(BL, 2, KE, 128)        # [b, dir, k, p]
        h0e = np.ascontiguousarray(
            (h0.transpose(3, 1, 2, 0) * SH).reshape(128, 2 * KE * BL)).astype(hdt)
        # decoder init h: [128, k*4+b] scaled
        ht = sty[lab[bs]].reshape(BL, KD, 128)
        ht0 = np.ascontiguousarray(
            (ht.transpose(2, 1, 0) * SH).reshape(128, KD * BL)).astype(hdt)
        in_maps.append(dict(
            embT_e=embT_e, embT_d=embT_d, h0e_q=h0e, ht0_q=ht0,
            wih_e=wih_e, wih_d=wih_d, whh_e=whh_e, whh_d=whh_d,
            wtr=wtr, wf1=wf1, wf2=wf2,
            bs_e=bs_e, bs_d=bs_d, b1a=b1a, b1h=b1h,
            descale=descale, idscale=idscale))
    return in_maps


_NC_CACHE = {}


def kernel(**inputs):
    bias_free = all(
        not np.any(np.asarray(inputs[k]))
        for k in ("bih_f", "bhh_f", "bih_b", "bhh_b", "bih_d", "bhh_d"))
    key = f"full{bias_free}"
    if key not in _NC_CACHE:
        _NC_CACHE[key] = build(bias_free=bias_free)
    nc = _NC_CACHE[key]
    in_maps = prep_inputs(inputs)
    res = run_bass_kernel_spmd(nc, in_maps, core_ids=list(range(N_CORES)))
    return np.concatenate(
        [r["out"].astype(np.float32) for r in res.results], axis=0)
